# revision 1
# baseline (speedup 1.0000x reference)
# Distributed Bass kernel for nn_DecoderBlock (AdaLN decoder block) on 8 TRN2 cores.
#
# Sharding: core i -> (batch b = i//4, sequence quarter r = i%4, 512 tokens).
# Weights replicated (bf16). The only collective is a 4-rank AllGather of the
# local K^T / V slices per batch group.
#
# Layout convention: every on-chip activation is stored transposed,
# [features(partitions), tokens(free)], so each linear y = h @ W uses the
# weight (in,out) directly as matmul lhsT and needs no on-chip transposes.
# Host pre-transposes/shards x and cond, folds 1/sqrt(d) into the q columns
# of qkv_w and the AdaLN "+1" into the gamma half of p1_b/p2_b.

import os

os.environ.setdefault("MYCRO_LOCAL_CACHE", "1")

import numpy as np
import ml_dtypes

import concourse.bass as bass
import concourse.mybir as mybir
import concourse.tile as tile
from concourse import bacc
from concourse.bass_utils import run_bass_kernel_spmd

F32 = mybir.dt.float32
BF16 = mybir.dt.bfloat16
AF = mybir.ActivationFunctionType
ALU = mybir.AluOpType

D = 1024        # d_model
DC = 512        # d_cond
H = 16          # heads
DH = 64         # head dim
FF = 4096       # ffn dim
T = 512         # tokens per core
S = 2048        # sequence length per batch
B = 2
NCORES = 8
GROUP = 4       # cores per batch group
EPS = 1e-5

_CACHE = {}


def _build(sim_safe=False):
    nc = bacc.Bacc(
        "TRN2",
        target_bir_lowering=False,
        debug=False,
        enable_asserts=False,
        num_devices=NCORES,
    )

    # ---- DRAM I/O ----
    xT = nc.dram_tensor("xT", [D, T], F32, kind="ExternalInput").ap()
    condT = nc.dram_tensor("condT", [DC, T], BF16, kind="ExternalInput").ap()
    p1w = nc.dram_tensor("p1w", [DC, 2 * D], BF16, kind="ExternalInput").ap()
    p1b = nc.dram_tensor("p1b", [128, 16], F32, kind="ExternalInput").ap()
    qkvw = nc.dram_tensor("qkvw", [D, 3 * D], BF16, kind="ExternalInput").ap()
    wo = nc.dram_tensor("wo", [D, D], BF16, kind="ExternalInput").ap()
    p2w = nc.dram_tensor("p2w", [DC, 2 * D], BF16, kind="ExternalInput").ap()
    p2b = nc.dram_tensor("p2b", [128, 16], F32, kind="ExternalInput").ap()
    w1 = nc.dram_tensor("w1", [D, FF], BF16, kind="ExternalInput").ap()
    b1 = nc.dram_tensor("b1", [128, 32], F32, kind="ExternalInput").ap()
    w2 = nc.dram_tensor("w2", [FF, D], BF16, kind="ExternalInput").ap()
    b2 = nc.dram_tensor("b2", [128, 8], F32, kind="ExternalInput").ap()
    out_d = nc.dram_tensor("out", [D, T], F32, kind="ExternalOutput").ap()

    with tile.TileContext(nc) as tc:
        _emit(nc, tc, xT, condT, p1w, p1b, qkvw, wo, p2w, p2b, w1, b1, w2, b2, out_d, sim_safe)

    nc.compile()
    return nc


def _emit(nc, tc, xT, condT, p1w, p1b, qkvw, wo, p2w, p2b, w1, b1, w2, b2, out_d, sim_safe):
    # Pool lifetimes follow a two-sided stack discipline (LIFO per side):
    # left = phase-nested pools, right = phase-crossing carries.
    def pool(name, bufs=1, space="SBUF", side=None):
        return tc.alloc_tile_pool(name=name, bufs=bufs, space=space, side=side)

    # ---------------- persistent pools ----------------
    const = pool("const")
    work = pool("work", bufs=4)            # [128,T] temporaries
    small = pool("small", bufs=4)          # [1,T] stats
    dram = pool("dram", bufs=1, space="DRAM")

    # right-side carries (alloc order = reverse of release order)
    x1_pool = pool("x1_pool", side="right")                    # lives to the end
    x1t_sb = x1_pool.tile([128, 8 * T], F32, name="x1t_sb")
    x_pool = pool("x_pool", side="right")                      # released after AdaLN1
    xT_sb = x_pool.tile([128, 8 * T], F32, name="xT_sb")

    # ---------------- constants ----------------
    ones_col_bf = const.tile([128, 1], BF16, name="ones_col_bf")
    nc.vector.memset(ones_col_bf[:], 1.0)
    ones_row_f = const.tile([1, 128], F32, name="ones_row_f")
    nc.vector.memset(ones_row_f[:], 1.0)
    eps_t = const.tile([1, 1], F32, name="eps_t")
    nc.vector.memset(eps_t[:], EPS)

    ones_all = const.tile([128, 64], F32, name="ones_all")
    nc.vector.memset(ones_all[:], 1.0)

    p1b_sb = const.tile([128, 16], F32, name="p1b_sb")
    nc.sync.dma_start(p1b_sb[:], p1b)
    p2b_sb = const.tile([128, 16], F32, name="p2b_sb")
    nc.sync.dma_start(p2b_sb[:], p2b)
    b1_sb = const.tile([128, 32], F32, name="b1_sb")
    nc.sync.dma_start(b1_sb[:], b1)
    b2_sb = const.tile([128, 8], F32, name="b2_sb")
    nc.sync.dma_start(b2_sb[:], b2)

    # ---------------- input loads (cond/p1w first: they gate the first ops) --
    cond_pool = pool("cond_pool")
    cond_sb = cond_pool.tile([128, 4 * T], BF16, name="cond_sb")
    for a in range(4):
        nc.sync.dma_start(cond_sb[:, T * a:T * (a + 1)], condT[128 * a:128 * (a + 1), :])
    sc_sb = cond_pool.tile([128, 4 * T], BF16, name="sc_sb")

    proj_pool = pool("proj_pool")
    p1w_sb = proj_pool.tile([128, 4 * 2048], BF16, name="p1w_sb")
    for a in range(4):
        nc.sync.dma_start(p1w_sb[:, 2048 * a:2048 * (a + 1)], p1w[128 * a:128 * (a + 1), :])
    p2w_sb = proj_pool.tile([128, 4 * 2048], BF16, name="p2w_sb")
    for a in range(4):
        nc.sync.dma_start(p2w_sb[:, 2048 * a:2048 * (a + 1)], p2w[128 * a:128 * (a + 1), :])

    for a in range(8):
        nc.sync.dma_start(xT_sb[:, T * a:T * (a + 1)], xT[128 * a:128 * (a + 1), :])

    qkvw_pool = pool("qkvw_pool")
    qkvw_sb = qkvw_pool.tile([128, 8 * 3072], BF16, name="qkvw_sb")
    for a in range(8):
        nc.sync.dma_start(qkvw_sb[:, 3072 * a:3072 * (a + 1)], qkvw[128 * a:128 * (a + 1), :])

    ps_a = pool("ps_a", bufs=2, space="PSUM")       # stats + broadcast banks
    mm_ps = pool("mm_ps", bufs=3, space="PSUM")     # matmul eviction banks

    # ---------------- helpers ----------------
    def adaln(src_f32, gb_sb, h_sb, tmp_pool, st_ps):
        """src_f32: [128, 8*T] f32 ([D, T] transposed); writes h_sb bf16."""
        src_bf = tmp_pool.tile([128, 8 * T], BF16, name="src_bf", tag="src_bf")
        nc.vector.tensor_copy(src_bf[:], src_f32[:])
        sq = tmp_pool.tile([128, 8 * T], BF16, name="sq", tag="sq")
        nc.vector.tensor_mul(sq[:], src_bf[:], src_bf[:])

        sums = st_ps.tile([1, T], F32, name="sums", tag="st")
        for j in range(8):
            nc.tensor.matmul(sums[:], ones_col_bf[:], src_bf[:, j * T:(j + 1) * T],
                             start=(j == 0), stop=(j == 7))
        sumsq = st_ps.tile([1, T], F32, name="sumsq", tag="st")
        for j in range(8):
            nc.tensor.matmul(sumsq[:], ones_col_bf[:], sq[:, j * T:(j + 1) * T],
                             start=(j == 0), stop=(j == 7))

        mu = small.tile([1, T], F32, name="mu", tag="sm")
        nc.vector.tensor_scalar_mul(mu[:], sums[:], 1.0 / D)
        musq = small.tile([1, T], F32, name="musq", tag="sm")
        nc.vector.tensor_mul(musq[:], mu[:], mu[:])
        var = small.tile([1, T], F32, name="var", tag="sm")
        nc.vector.scalar_tensor_tensor(var[:], sumsq[:], 1.0 / D, musq[:],
                                       op0=ALU.mult, op1=ALU.subtract)
        lnv = small.tile([1, T], F32, name="lnv", tag="sm")
        nc.scalar.activation(lnv[:], var[:], AF.Ln, bias=eps_t[:], scale=1.0)
        rs = small.tile([1, T], F32, name="rs", tag="sm")
        nc.scalar.activation(rs[:], lnv[:], AF.Exp, scale=-0.5)

        mu_b = st_ps.tile([128, T], F32, name="mu_b", tag="bc")
        nc.tensor.matmul(mu_b[:], ones_row_f[:], mu[:], start=True, stop=True)
        rs_b = st_ps.tile([128, T], F32, name="rs_b", tag="bc")
        nc.tensor.matmul(rs_b[:], ones_row_f[:], rs[:], start=True, stop=True)
        mu_bs = work.tile([128, T], BF16, name="mu_bs", tag="wk")
        nc.vector.tensor_copy(mu_bs[:], mu_b[:])
        rs_bs = work.tile([128, T], BF16, name="rs_bs", tag="wk")
        nc.vector.tensor_copy(rs_bs[:], rs_b[:])

        for j in range(8):
            sl = slice(j * T, (j + 1) * T)
            t1 = work.tile([128, T], BF16, name="t1", tag="wk")
            nc.vector.tensor_sub(t1[:], src_bf[:, sl], mu_bs[:])
            t2 = work.tile([128, T], BF16, name="t2", tag="wk")
            nc.vector.tensor_mul(t2[:], t1[:], rs_bs[:])
            t3 = work.tile([128, T], BF16, name="t3", tag="wk")
            nc.vector.tensor_mul(t3[:], t2[:], gb_sb[:, sl])           # *(1+gamma)
            nc.vector.tensor_add(h_sb[:, sl], t3[:], gb_sb[:, (8 + j) * T:(9 + j) * T])

    def proj_gb(w_sb, b_sb, gb_sb, m_ps):
        """gb^T = (silu(cond) @ W + b)^T : 16 M-tiles of [128, T]."""
        for m in range(16):
            ps = m_ps.tile([128, T], F32, name="gbps", tag="mm")
            for k in range(4):
                nc.tensor.matmul(ps[:], w_sb[:, 2048 * k + 128 * m: 2048 * k + 128 * (m + 1)],
                                 sc_sb[:, k * T:(k + 1) * T],
                                 start=(k == 0), stop=(k == 3))
            nc.vector.tensor_scalar_add(gb_sb[:, m * T:(m + 1) * T], ps[:],
                                        b_sb[:, m:m + 1])

    # ---------------- AdaLN 1 ----------------
    # silu(c) = c * sigmoid(c), per tile so proj matmuls start early
    sig_sb = cond_pool.tile([128, 4 * T], BF16, name="sig_sb")
    for a in range(4):
        sl = slice(T * a, T * (a + 1))
        nc.scalar.activation(sig_sb[:, sl], cond_sb[:, sl], AF.Sigmoid)
        nc.vector.tensor_mul(sc_sb[:, sl], cond_sb[:, sl], sig_sb[:, sl])

    h1_pool = pool("h1_pool")
    h1_sb = h1_pool.tile([128, 8 * T], BF16, name="h1_sb")

    gb1_pool = pool("gb1_pool")
    gb1_sb = gb1_pool.tile([128, 16 * T], BF16, name="gb1_sb")
    proj_gb(p1w_sb, p1b_sb, gb1_sb, mm_ps)

    aln1_tmp = pool("aln1_tmp")
    adaln(xT_sb, gb1_sb, h1_sb, aln1_tmp, ps_a)
    aln1_tmp.release()
    gb1_pool.release()
    x_pool.release()

    # ---------------- qkv + chunked collective ----------------
    # 4 AllGather chunks, one per 4-head group: chunk c carries k^T feature
    # rows [256c:256c+256] and v columns [256c:256c+256], so attention on
    # head-pairs 2c,2c+1 can start while later chunks are still in flight.
    kv_pool = pool("kv_pool")
    kT_loc = kv_pool.tile([128, 8 * T], BF16, name="kT_loc")
    v_loc = kv_pool.tile([128, 4 * D], BF16, name="v_loc")

    NCH = 4
    kv_ins = [dram.tile([512, T], BF16, name=f"kv_in{c}") for c in range(NCH)]
    kv_outs = [dram.tile([GROUP, 512, T], BF16, name=f"kv_out{c}") for c in range(NCH)]

    for c in range(NCH):
        # k^T feature M-tiles for heads 4c..4c+3
        for ml in range(2):
            m = 8 + 2 * c + ml
            ps = mm_ps.tile([128, T], F32, name="kps", tag="mm")
            for k in range(8):
                nc.tensor.matmul(ps[:], qkvw_sb[:, 3072 * k + 128 * m: 3072 * k + 128 * (m + 1)],
                                 h1_sb[:, k * T:(k + 1) * T],
                                 start=(k == 0), stop=(k == 7))
            nc.vector.tensor_copy(kT_loc[:, (m - 8) * T:(m - 7) * T], ps[:])
        # v quarter c ([tokens, 256 features]), token M-tiles
        for mt in range(4):
            ps = mm_ps.tile([128, 256], F32, name="vps", tag="mm")
            for k in range(8):
                nc.tensor.matmul(
                    ps[:],
                    h1_sb[:, k * T + 128 * mt: k * T + 128 * (mt + 1)],
                    qkvw_sb[:, 3072 * k + 2048 + 256 * c: 3072 * k + 2048 + 256 * (c + 1)],
                    start=(k == 0), stop=(k == 7))
            nc.vector.tensor_copy(v_loc[:, 1024 * mt + 256 * c: 1024 * mt + 256 * (c + 1)], ps[:])
        # bounce writes + collective for this chunk
        for ml in range(2):
            nc.sync.dma_start(kv_ins[c][128 * ml:128 * (ml + 1), :],
                              kT_loc[:, (2 * c + ml) * T:(2 * c + ml + 1) * T])
        vdst = kv_ins[c][256:512, :].rearrange("r (two f) -> (r two) f", two=2)
        nc.sync.dma_start(vdst.rearrange("(m p) f -> p m f", m=4),
                          v_loc.rearrange("p (m f) -> p m f", m=4)[:, :, 256 * c:256 * (c + 1)])
        nc.gpsimd.collective_compute(
            "AllGather",
            ALU.bypass,
            replica_groups=[[0, 1, 2, 3], [4, 5, 6, 7]],
            ins=[kv_ins[c][:]],
            outs=[kv_outs[c][:]],
        )
    kv_pool.release()

    # right-side carries for the attention phase
    gb2_pool = pool("gb2_pool", side="right")
    gb2_sb = gb2_pool.tile([128, 16 * T], BF16, name="gb2_sb")
    oT_pool = pool("oT_pool", side="right")
    oT_sb = oT_pool.tile([128, 8 * T], BF16, name="oT_sb")
    q_pool = pool("q_pool", side="right")
    qT_sb = q_pool.tile([128, 8 * T], BF16, name="qT_sb")

    # q^T (feature M-tiles 0..7), overlaps with collective
    for m in range(8):
        ps = mm_ps.tile([128, T], F32, name="qps", tag="mm")
        for k in range(8):
            nc.tensor.matmul(ps[:], qkvw_sb[:, 3072 * k + 128 * m: 3072 * k + 128 * (m + 1)],
                             h1_sb[:, k * T:(k + 1) * T],
                             start=(k == 0), stop=(k == 7))
        nc.vector.tensor_copy(qT_sb[:, m * T:(m + 1) * T], ps[:])

    # gb2 projection, overlaps with collective
    proj_gb(p2w_sb, p2b_sb, gb2_sb, mm_ps)

    h1_pool.release()
    qkvw_pool.release()
    proj_pool.release()
    cond_pool.release()
    mm_ps.release()
    ps_a.release()

    # ---------------- attention ----------------
    att_pool = pool("att_pool")
    kT_full = att_pool.tile([128, 8 * S], BF16, name="kT_full")
    VW = DH + 1  # 65: per-head V columns + ones column (softmax denominator)
    vext = att_pool.tile([128, 16 * H * VW], BF16, name="vext")
    vext_v = vext.rearrange("p (c h m) -> p c h m", c=16, m=VW)
    nc.vector.memset(vext_v[:, :, :, DH:DH + 1], 1.0)

    def readback_chunk(c):
        for fl in range(2):
            f = 2 * c + fl
            for r in range(GROUP):
                nc.sync.dma_start(kT_full[:, 2048 * f + 512 * r: 2048 * f + 512 * (r + 1)],
                                  kv_outs[c][r, 128 * fl:128 * (fl + 1), :])
        for r in range(GROUP):
            vch = kv_outs[c][r, 256:512, :].rearrange("q (two f) -> (q two) f", two=2)
            for lc in range(4):
                c2 = 4 * r + lc
                src = vch[128 * lc:128 * (lc + 1), :].rearrange("t (h d) -> t h d", d=DH)
                # SWDGE queue: keeps vext readbacks off the sync DMA queues so
                # they don't serialize behind later chunks' waits
                nc.gpsimd.dma_start(vext_v[:, c2, 4 * c:4 * (c + 1), 0:DH], src)

    p_pool = pool("p_pool", bufs=4)
    norm_pool = pool("norm_pool", bufs=2)
    sc_ps = pool("sc_ps", bufs=2, space="PSUM")     # [128,1024] = 2 banks each
    o_ps_pool = pool("o_ps", bufs=2, space="PSUM")

    norm_pending = []

    def after_av(pv_hp, o_tiles):
        # Part A (DVE only): evict raw o^T + denominator (freeing o psum
        # quickly) and compute the reciprocal; the PE-side broadcast runs a
        # pair later via flush_norm so the slow reciprocal never stalls PE.
        for hh in range(2):
            nc.vector.tensor_copy(oT_sb[64 * hh:64 * (hh + 1), pv_hp * T:(pv_hp + 1) * T],
                                  o_tiles[hh][0:DH, :])
            den = norm_pool.tile([128, T], F32, name="den", tag="den")
            nc.vector.tensor_copy(den[64:65, :], o_tiles[hh][DH:DH + 1, :])
            rec = norm_pool.tile([128, T], F32, name="rec", tag="rec", bufs=4)
            nc.vector.reciprocal(rec[64:65, :], den[64:65, :])
            norm_pending.append((pv_hp, hh, rec))

    def flush_norm():
        for (php, phh, rec) in norm_pending:
            rb = o_ps_pool.tile([64, T], F32, name="rb", tag="rb", bufs=2)
            nc.tensor.matmul(rb[:], ones_all[64:65, :], rec[64:65, :], start=True, stop=True)
            rb_sb = norm_pool.tile([128, T], BF16, name="rb_sb", tag="rbs")
            nc.vector.tensor_copy(rb_sb[64 * phh:64 * (phh + 1), :], rb[:])
            osl = oT_sb[64 * phh:64 * (phh + 1), php * T:(php + 1) * T]
            nc.vector.tensor_mul(osl, osl, rb_sb[64 * phh:64 * (phh + 1), :])
        norm_pending.clear()

    prev = None
    for hp in range(8):
        if hp % 2 == 0:
            readback_chunk(hp // 2)
        p_tiles = [p_pool.tile([128, 16 * T], BF16, name=f"pt{hh}", tag="p") for hh in range(2)]
        q_h = [qT_sb[64 * hh:64 * (hh + 1), hp * T:(hp + 1) * T] for hh in range(2)]
        o_tiles = None
        if prev is not None:
            o_tiles = [o_ps_pool.tile([128, T], F32, name="o_ps", tag="o") for _ in range(2)]
        # 8 groups: scores for chunks (2m2, 2m2+1) of both heads, interleaved
        # with 4 AV matmuls of the previous pair so PE work overlaps ACT exp.
        for m2 in range(8):
            scts = [sc_ps.tile([128, 1024], F32, name="sct", tag="s") for _ in range(2)]
            for half in range(2):
                m = 2 * m2 + half
                for hh in range(2):
                    rows = slice(64 * hh, 64 * (hh + 1))
                    nc.tensor.matmul(
                        scts[hh][:, 512 * half:512 * (half + 1)],
                        kT_full[rows, 2048 * hp + 128 * m: 2048 * hp + 128 * (m + 1)],
                        q_h[hh],
                        start=True, stop=True)
            if prev is not None:
                pv_tiles, pv_hp = prev
                for hh in range(2):
                    h = 2 * pv_hp + hh
                    for half in range(2):
                        cc = 2 * m2 + half
                        nc.tensor.matmul(
                            o_tiles[hh][0:VW, :],
                            vext[:, VW * (16 * cc + h): VW * (16 * cc + h) + VW],
                            pv_tiles[hh][:, cc * T:(cc + 1) * T],
                            start=(cc == 0), stop=(cc == 15))
            for hh in range(2):
                nc.scalar.activation(p_tiles[hh][:, 2 * m2 * T:(2 * m2 + 2) * T],
                                     scts[hh][:], AF.Exp)
        flush_norm()
        if prev is not None:
            after_av(prev[1], o_tiles)
        prev = (p_tiles, hp)

    # tail: AV + normalize for the last pair
    pv_tiles, pv_hp = prev
    o_tiles = [o_ps_pool.tile([128, T], F32, name="o_ps", tag="o") for _ in range(2)]
    for cc in range(16):
        for hh in range(2):
            h = 2 * pv_hp + hh
            nc.tensor.matmul(
                o_tiles[hh][0:VW, :],
                vext[:, VW * (16 * cc + h): VW * (16 * cc + h) + VW],
                pv_tiles[hh][:, cc * T:(cc + 1) * T],
                start=(cc == 0), stop=(cc == 15))
    flush_norm()
    after_av(pv_hp, o_tiles)
    flush_norm()

    o_ps_pool.release()
    sc_ps.release()
    norm_pool.release()
    p_pool.release()
    att_pool.release()
    q_pool.release()

    # ---------------- attn_out + residual ----------------
    ps_b = pool("ps_b", bufs=2, space="PSUM")
    mm_ps2 = pool("mm_ps2", bufs=3, space="PSUM")

    wo_pool = pool("wo_pool")
    wo_sb = wo_pool.tile([128, 8 * D], BF16, name="wo_sb")
    for a in range(8):
        nc.sync.dma_start(wo_sb[:, 1024 * a:1024 * (a + 1)], wo[128 * a:128 * (a + 1), :])

    xre_pool = pool("xre_pool", side="right")
    xre_sb = xre_pool.tile([128, 8 * T], F32, name="xre_sb")
    for a in range(8):
        nc.sync.dma_start(xre_sb[:, T * a:T * (a + 1)], xT[128 * a:128 * (a + 1), :])

    for m in range(8):
        ps = mm_ps2.tile([128, T], F32, name="aops", tag="mm")
        for k in range(8):
            nc.tensor.matmul(ps[:], wo_sb[:, 1024 * k + 128 * m: 1024 * k + 128 * (m + 1)],
                             oT_sb[:, k * T:(k + 1) * T],
                             start=(k == 0), stop=(k == 7))
        nc.vector.tensor_add(x1t_sb[:, m * T:(m + 1) * T], ps[:], xre_sb[:, m * T:(m + 1) * T])
    wo_pool.release()
    xre_pool.release()
    oT_pool.release()

    # ---------------- AdaLN 2 ----------------
    g_pool = pool("g_pool")
    g_sb = g_pool.tile([128, 32 * T], BF16, name="g_sb")

    h2_pool = pool("h2_pool")
    h2_sb = h2_pool.tile([128, 8 * T], BF16, name="h2_sb")

    w1_pool = pool("w1_pool")
    w1_sb = w1_pool.tile([128, 8 * FF], BF16, name="w1_sb")
    for a in range(8):
        nc.sync.dma_start(w1_sb[:, 4096 * a:4096 * (a + 1)], w1[128 * a:128 * (a + 1), :])

    aln2_tmp = pool("aln2_tmp")
    adaln(x1t_sb, gb2_sb, h2_sb, aln2_tmp, ps_b)
    aln2_tmp.release()
    gb2_pool.release()

    # ---------------- FFN ----------------
    for m in range(32):
        ps = mm_ps2.tile([128, T], F32, name="f1ps", tag="mm")
        for k in range(8):
            nc.tensor.matmul(ps[:], w1_sb[:, 4096 * k + 128 * m: 4096 * k + 128 * (m + 1)],
                             h2_sb[:, k * T:(k + 1) * T],
                             start=(k == 0), stop=(k == 7))
        if sim_safe:
            # sim has no Gelu: x*sigmoid(1.702x) approximation
            u = work.tile([128, T], F32, name="u", tag="wk32f")
            nc.vector.tensor_scalar_add(u[:], ps[:], b1_sb[:, m:m + 1])
            sg = work.tile([128, T], BF16, name="sg", tag="wk")
            nc.scalar.activation(sg[:], u[:], AF.Sigmoid, scale=1.702)
            nc.vector.tensor_mul(g_sb[:, m * T:(m + 1) * T], u[:], sg[:])
        else:
            nc.scalar.activation(g_sb[:, m * T:(m + 1) * T], ps[:], AF.Gelu,
                                 bias=b1_sb[:, m:m + 1], scale=1.0)
    w1_pool.release()
    h2_pool.release()
    mm_ps2.release()
    ps_b.release()

    # ffn2: k-outer, stream w2 k-tiles; two m-halves so the first half's
    # evictions overlap the second half's matmuls
    w2_pool = pool("w2_pool", bufs=4)
    ff2_ps = pool("ff2_ps", bufs=1, space="PSUM")
    out_pool0 = pool("out_pool0")
    out_sb = out_pool0.tile([128, 8 * T], F32, name="out_sb")
    for half in range(2):
        o2 = [ff2_ps.tile([128, T], F32, name=f"ff2_{m}", tag=f"ff2_{m}") for m in range(4)]
        for k in range(32):
            w2t = w2_pool.tile([128, 512], BF16, name="w2t", tag="w2t")
            nc.sync.dma_start(w2t[:], w2[128 * k: 128 * (k + 1), 512 * half:512 * (half + 1)])
            for m in range(4):
                nc.tensor.matmul(o2[m][:], w2t[:, 128 * m: 128 * (m + 1)],
                                 g_sb[:, k * T:(k + 1) * T],
                                 start=(k == 0), stop=(k == 31))
        for m in range(4):
            gm = 4 * half + m
            nc.vector.scalar_tensor_tensor(out_sb[:, gm * T:(gm + 1) * T], o2[m][:],
                                           b2_sb[:, gm:gm + 1], x1t_sb[:, gm * T:(gm + 1) * T],
                                           op0=ALU.add, op1=ALU.add)

    for a in range(8):
        nc.sync.dma_start(out_d[128 * a:128 * (a + 1), :], out_sb[:, T * a:T * (a + 1)])

    out_pool0.release()
    w2_pool.release()
    g_pool.release()
    ff2_ps.release()
    x1_pool.release()
    small.release()
    work.release()
    const.release()
    dram.release()


def _bf16(a):
    return np.ascontiguousarray(a).astype(ml_dtypes.bfloat16)


def _prep_maps(x, cond, p1_w, p1_b, qkv_w, attn_out_w, p2_w, p2_b,
               ffn_w1, ffn_b1, ffn_w2, ffn_b2):
    x = np.asarray(x, np.float32)
    cond = np.asarray(cond, np.float32)
    qkv_mod = np.asarray(qkv_w, np.float32).copy()
    qkv_mod[:, :D] *= DH ** -0.5                      # fold 1/sqrt(d) into q
    p1b_mod = np.asarray(p1_b, np.float32).copy()
    p1b_mod[:D] += 1.0                                # fold AdaLN "+1" into gamma bias
    p2b_mod = np.asarray(p2_b, np.float32).copy()
    p2b_mod[:D] += 1.0

    shared = {
        "p1w": _bf16(p1_w),
        "p1b": np.ascontiguousarray(p1b_mod.reshape(16, 128).T, np.float32),
        "qkvw": _bf16(qkv_mod),
        "wo": _bf16(attn_out_w),
        "p2w": _bf16(p2_w),
        "p2b": np.ascontiguousarray(p2b_mod.reshape(16, 128).T, np.float32),
        "w1": _bf16(ffn_w1),
        "b1": np.ascontiguousarray(np.asarray(ffn_b1, np.float32).reshape(32, 128).T,
                                   np.float32),
        "w2": _bf16(ffn_w2),
        "b2": np.ascontiguousarray(np.asarray(ffn_b2, np.float32).reshape(8, 128).T,
                                   np.float32),
    }
    in_maps = []
    for core in range(NCORES):
        b, r = core // GROUP, core % GROUP
        sl = slice(T * r, T * (r + 1))
        m = dict(shared)
        m["xT"] = np.ascontiguousarray(x[b, sl, :].T, np.float32)
        m["condT"] = _bf16(cond[b, sl, :].T)
        in_maps.append(m)
    return in_maps


def _get_nc():
    if "nc" not in _CACHE:
        _CACHE["nc"] = _build()
    return _CACHE["nc"]


def _install_ntff_hook():
    """This image's antenv lacks axon_hooks; recreate it (see trn_boot.py)."""
    import sys, types, ctypes, contextlib

    if "antenv.axon_hooks" in sys.modules:
        return
    mod = types.ModuleType("antenv.axon_hooks")
    state = {"hook": None}
    mod.set_axon_ntff_profile_hook = lambda h: state.__setitem__("hook", h)
    mod.get_axon_ntff_profile_hook = lambda: state["hook"]
    sys.modules["antenv.axon_hooks"] = mod
    try:
        import antenv
        antenv.axon_hooks = mod
    except ImportError:
        pass

    so_path = "/opt/axon/libaxon_pjrt.so"
    if not os.path.exists(so_path):
        return
    lib = ctypes.CDLL(so_path)
    if not hasattr(lib, "axon_start_nrt_profile"):
        return
    lib.axon_start_nrt_profile.argtypes = [ctypes.POINTER(ctypes.c_int64), ctypes.c_size_t]
    lib.axon_start_nrt_profile.restype = ctypes.c_int64
    lib.axon_stop_nrt_profile.argtypes = [ctypes.c_char_p]
    lib.axon_stop_nrt_profile.restype = ctypes.c_int64

    @contextlib.contextmanager
    def _hook(output_dir, device_ids):
        import jax
        jax.devices()
        if device_ids:
            ids = (ctypes.c_int64 * len(device_ids))(*device_ids)
            rc = lib.axon_start_nrt_profile(ids, len(device_ids))
        else:
            rc = lib.axon_start_nrt_profile(None, 0)
        if rc != 0:
            raise RuntimeError(f"axon_start_nrt_profile rc={rc}")
        try:
            yield
        finally:
            n = lib.axon_stop_nrt_profile(str(output_dir).encode())
            print(f"ntff profile: {n} file(s) -> {output_dir}")

    mod.set_axon_ntff_profile_hook(_hook)


def run(in_maps, trace=False, **kw):
    if trace:
        _install_ntff_hook()
    nc = _get_nc()
    return run_bass_kernel_spmd(nc, in_maps, core_ids=list(range(NCORES)),
                                trace=trace, **kw)


def kernel(**inputs):
    in_maps = _prep_maps(**inputs)
    res = run(in_maps).results
    out = np.empty((B, S, D), np.float32)
    for core in range(NCORES):
        b, r = core // GROUP, core % GROUP
        out[b, T * r: T * (r + 1), :] = res[core]["out"].T
    return out



# revision 13
# speedup vs baseline: 1.0006x; 1.0006x over previous
# Distributed Bass kernel for nn_DecoderBlock (AdaLN decoder block) on 8 TRN2 cores.
#
# Sharding: core i -> (batch b = i//4, sequence quarter r = i%4, 512 tokens).
# Weights replicated (bf16). The only collective is a 4-rank AllGather of the
# local K^T / V slices per batch group.
#
# Layout convention: every on-chip activation is stored transposed,
# [features(partitions), tokens(free)], so each linear y = h @ W uses the
# weight (in,out) directly as matmul lhsT and needs no on-chip transposes.
# Host pre-transposes/shards x and cond, folds 1/sqrt(d) into the q columns
# of qkv_w and the AdaLN "+1" into the gamma half of p1_b/p2_b.
#
# Engine budget per phase (per core): pre-attention is PE-bound (~77us of
# matmul), attention is ACT-bound (16.8M softmax exps ~= 147us), FFN tail is
# PE-bound (~110us). AdaLN stats matmuls interleave with the gamma-half
# projection matmuls so the LayerNorm chain never serializes the PE; gb/k/v
# PSUM evictions run on the otherwise-idle ACT engine pre-attention; softmax
# normalization uses reciprocal_approx_fast straight off the PSUM denominator
# row plus a single K=2 selector-broadcast matmul per head pair.

import os

os.environ.setdefault("MYCRO_LOCAL_CACHE", "1")

import numpy as np
import ml_dtypes

import concourse.bass as bass
import concourse.mybir as mybir
import concourse.tile as tile
from concourse import bacc
from concourse.bass_utils import run_bass_kernel_spmd

F32 = mybir.dt.float32
BF16 = mybir.dt.bfloat16
AF = mybir.ActivationFunctionType
ALU = mybir.AluOpType

D = 1024        # d_model
DC = 512        # d_cond
H = 16          # heads
DH = 64         # head dim
FF = 4096       # ffn dim
T = 512         # tokens per core
S = 2048        # sequence length per batch
B = 2
NCORES = 8
GROUP = 4       # cores per batch group
EPS = 1e-5

_CACHE = {}


def _build(sim_safe=False):
    nc = bacc.Bacc(
        "TRN2",
        target_bir_lowering=False,
        debug=False,
        enable_asserts=False,
        num_devices=NCORES,
    )

    # ---- DRAM I/O ----
    xT = nc.dram_tensor("xT", [D, T], F32, kind="ExternalInput").ap()
    condT = nc.dram_tensor("condT", [DC, T], BF16, kind="ExternalInput").ap()
    p1w = nc.dram_tensor("p1w", [DC, 2 * D], BF16, kind="ExternalInput").ap()
    p1b = nc.dram_tensor("p1b", [128, 16], F32, kind="ExternalInput").ap()
    qkvw = nc.dram_tensor("qkvw", [D, 3 * D], BF16, kind="ExternalInput").ap()
    wo = nc.dram_tensor("wo", [D, D], BF16, kind="ExternalInput").ap()
    p2w = nc.dram_tensor("p2w", [DC, 2 * D], BF16, kind="ExternalInput").ap()
    p2b = nc.dram_tensor("p2b", [128, 16], F32, kind="ExternalInput").ap()
    w1 = nc.dram_tensor("w1", [D, FF], BF16, kind="ExternalInput").ap()
    b1 = nc.dram_tensor("b1", [128, 32], F32, kind="ExternalInput").ap()
    w2 = nc.dram_tensor("w2", [FF, D], BF16, kind="ExternalInput").ap()
    b2 = nc.dram_tensor("b2", [128, 8], F32, kind="ExternalInput").ap()
    out_d = nc.dram_tensor("out", [D, T], F32, kind="ExternalOutput").ap()

    with tile.TileContext(nc) as tc:
        _emit(nc, tc, xT, condT, p1w, p1b, qkvw, wo, p2w, p2b, w1, b1, w2, b2, out_d, sim_safe)

    nc.compile()
    return nc


def _emit(nc, tc, xT, condT, p1w, p1b, qkvw, wo, p2w, p2b, w1, b1, w2, b2, out_d, sim_safe):
    # Pool lifetimes follow a two-sided stack discipline (LIFO per side):
    # left = phase-nested pools, right = phase-crossing carries.
    def pool(name, bufs=1, space="SBUF", side=None):
        return tc.alloc_tile_pool(name=name, bufs=bufs, space=space, side=side)

    # ---------------- persistent pools ----------------
    const = pool("const")
    work = pool("work", bufs=4)            # [128,T] temporaries
    small = pool("small", bufs=4)          # [1,T] stats
    dram = pool("dram", bufs=1, space="DRAM")

    # right-side carries (alloc order = reverse of release order)
    x1_pool = pool("x1_pool", side="right")                    # lives to the end
    x1t_sb = x1_pool.tile([128, 8 * T], F32, name="x1t_sb")
    x_pool = pool("x_pool", side="right")                      # released after AdaLN1
    xT_sb = x_pool.tile([128, 8 * T], F32, name="xT_sb")

    # ---------------- constants ----------------
    ones_col_bf = const.tile([128, 1], BF16, name="ones_col_bf")
    nc.vector.memset(ones_col_bf[:], 1.0)
    ones_row_f = const.tile([1, 128], F32, name="ones_row_f")
    nc.vector.memset(ones_row_f[:], 1.0)
    eps_t = const.tile([1, 1], F32, name="eps_t")
    nc.vector.memset(eps_t[:], EPS)

    # head-pair selectors: rb[p,:] = recA for p<64, recB for p>=64
    selA = const.tile([1, 128], F32, name="selA")
    nc.vector.memset(selA[:], 0.0)
    nc.vector.memset(selA[0:1, 0:64], 1.0)
    selB = const.tile([1, 128], F32, name="selB")
    nc.vector.memset(selB[:], 0.0)
    nc.vector.memset(selB[0:1, 64:128], 1.0)

    p1b_sb = const.tile([128, 16], F32, name="p1b_sb")
    nc.sync.dma_start(p1b_sb[:], p1b)
    p2b_sb = const.tile([128, 16], F32, name="p2b_sb")
    nc.sync.dma_start(p2b_sb[:], p2b)
    b1_sb = const.tile([128, 32], F32, name="b1_sb")
    nc.sync.dma_start(b1_sb[:], b1)
    b2_sb = const.tile([128, 8], F32, name="b2_sb")
    nc.sync.dma_start(b2_sb[:], b2)

    # ---------------- input loads (cond/p1w first: they gate the first ops) --
    cond_pool = pool("cond_pool")
    cond_sb = cond_pool.tile([128, 4 * T], BF16, name="cond_sb")
    for a in range(4):
        nc.sync.dma_start(cond_sb[:, T * a:T * (a + 1)], condT[128 * a:128 * (a + 1), :])
    sc_sb = cond_pool.tile([128, 4 * T], BF16, name="sc_sb")

    proj_pool = pool("proj_pool")
    p1w_sb = proj_pool.tile([128, 4 * 2048], BF16, name="p1w_sb")
    for a in range(4):
        nc.sync.dma_start(p1w_sb[:, 2048 * a:2048 * (a + 1)], p1w[128 * a:128 * (a + 1), :])

    for a in range(8):
        nc.sync.dma_start(xT_sb[:, T * a:T * (a + 1)], xT[128 * a:128 * (a + 1), :])

    p2w_sb = proj_pool.tile([128, 4 * 2048], BF16, name="p2w_sb")
    for a in range(4):
        nc.sync.dma_start(p2w_sb[:, 2048 * a:2048 * (a + 1)], p2w[128 * a:128 * (a + 1), :])

    qkvw_pool = pool("qkvw_pool")
    qkvw_sb = qkvw_pool.tile([128, 8 * 3072], BF16, name="qkvw_sb")
    for a in range(8):
        nc.sync.dma_start(qkvw_sb[:, 3072 * a:3072 * (a + 1)], qkvw[128 * a:128 * (a + 1), :])

    ps_a = pool("ps_a", bufs=2, space="PSUM")       # stats + broadcast banks
    mm_ps = pool("mm_ps", bufs=3, space="PSUM")     # matmul eviction banks

    # ---------------- AdaLN helpers ----------------
    def stats_finish(sums, sumsq, st_ps):
        """[1,T] PSUM sums -> broadcast bf16 mu/rs tiles (PE+ACT+DVE smalls)."""
        mu = small.tile([1, T], F32, name="mu", tag="sm")
        nc.vector.tensor_scalar_mul(mu[:], sums[:], 1.0 / D)
        musq = small.tile([1, T], F32, name="musq", tag="sm")
        nc.vector.tensor_mul(musq[:], mu[:], mu[:])
        var = small.tile([1, T], F32, name="var", tag="sm")
        nc.vector.scalar_tensor_tensor(var[:], sumsq[:], 1.0 / D, musq[:],
                                       op0=ALU.mult, op1=ALU.subtract)
        rs = small.tile([1, T], F32, name="rs", tag="sm")
        sd = small.tile([1, T], F32, name="sd", tag="sm")
        nc.scalar.activation(sd[:], var[:], AF.Sqrt, bias=eps_t[:], scale=1.0)
        if sim_safe:
            nc.vector.reciprocal(rs[:], sd[:])
        else:
            nc.vector.reciprocal_approx_fast(rs[:], sd[:])

        mu_b = st_ps.tile([128, T], F32, name="mu_b", tag="bc")
        nc.tensor.matmul(mu_b[:], ones_row_f[:], mu[:], start=True, stop=True)
        rs_b = st_ps.tile([128, T], F32, name="rs_b", tag="bc")
        nc.tensor.matmul(rs_b[:], ones_row_f[:], rs[:], start=True, stop=True)
        mu_bs = work.tile([128, T], BF16, name="mu_bs", tag="wk")
        nc.vector.tensor_copy(mu_bs[:], mu_b[:])
        rs_bs = work.tile([128, T], BF16, name="rs_bs", tag="wk")
        nc.vector.tensor_copy(rs_bs[:], rs_b[:])
        return mu_bs, rs_bs

    def normalize_tile(j, src_bf, gb_sb, h_sb, mu_bs, rs_bs):
        sl = slice(j * T, (j + 1) * T)
        t1 = work.tile([128, T], BF16, name="t1", tag="wk")
        nc.vector.tensor_sub(t1[:], src_bf[:, sl], mu_bs[:])
        t2 = work.tile([128, T], BF16, name="t2", tag="wk")
        nc.vector.tensor_mul(t2[:], t1[:], rs_bs[:])
        t3 = work.tile([128, T], BF16, name="t3", tag="wk")
        nc.vector.tensor_mul(t3[:], t2[:], gb_sb[:, sl])           # *(1+gamma)
        nc.vector.tensor_add(h_sb[:, sl], t3[:], gb_sb[:, (8 + j) * T:(9 + j) * T])

    def gb_mtile(w_sb, b_sb, gb_sb, m, m_ps, evict_act=True):
        ps = m_ps.tile([128, T], F32, name="gbps", tag="mm")
        for k in range(4):
            nc.tensor.matmul(ps[:], w_sb[:, 2048 * k + 128 * m: 2048 * k + 128 * (m + 1)],
                             sc_sb[:, k * T:(k + 1) * T],
                             start=(k == 0), stop=(k == 3))
        if evict_act and not sim_safe:
            nc.scalar.activation(gb_sb[:, m * T:(m + 1) * T], ps[:], AF.Identity,
                                 bias=b_sb[:, m:m + 1], scale=1.0)
        else:
            nc.vector.tensor_scalar_add(gb_sb[:, m * T:(m + 1) * T], ps[:],
                                        b_sb[:, m:m + 1])

    # ---------------- AdaLN 1 ----------------
    # silu(c) = c * sigmoid(c), per tile so proj matmuls start early
    sig_sb = cond_pool.tile([128, 4 * T], BF16, name="sig_sb")
    for a in range(4):
        sl = slice(T * a, T * (a + 1))
        nc.scalar.activation(sig_sb[:, sl], cond_sb[:, sl], AF.Sigmoid)
        nc.vector.tensor_mul(sc_sb[:, sl], cond_sb[:, sl], sig_sb[:, sl])

    h1_pool = pool("h1_pool")
    h1_sb = h1_pool.tile([128, 8 * T], BF16, name="h1_sb")

    gb1_pool = pool("gb1_pool")
    gb1_sb = gb1_pool.tile([128, 16 * T], BF16, name="gb1_sb")

    aln1_tmp = pool("aln1_tmp")
    src1_bf = aln1_tmp.tile([128, 8 * T], BF16, name="src1_bf")

    sums1 = ps_a.tile([1, T], F32, name="sums1", tag="st")
    sumsq1 = ps_a.tile([1, T], F32, name="sumsq1", tag="st")
    # gamma m-tiles interleaved with the x stats stream (PE never idles on DVE)
    for j in range(8):
        sl = slice(j * T, (j + 1) * T)
        nc.vector.tensor_copy(src1_bf[:, sl], xT_sb[:, sl])
        sqj = work.tile([128, T], BF16, name="sqj", tag="wk")
        nc.vector.tensor_mul(sqj[:], src1_bf[:, sl], src1_bf[:, sl])
        gb_mtile(p1w_sb, p1b_sb, gb1_sb, j, mm_ps)
        nc.tensor.matmul(sums1[:], ones_col_bf[:], src1_bf[:, sl],
                         start=(j == 0), stop=(j == 7))
        nc.tensor.matmul(sumsq1[:], ones_col_bf[:], sqj[:],
                         start=(j == 0), stop=(j == 7))
    # beta m-tiles; the mu/rs scalar chain overlaps these on ACT/DVE
    for j in range(8):
        gb_mtile(p1w_sb, p1b_sb, gb1_sb, 8 + j, mm_ps)
    mu_bs1, rs_bs1 = stats_finish(sums1, sumsq1, ps_a)
    for j in range(8):
        normalize_tile(j, src1_bf, gb1_sb, h1_sb, mu_bs1, rs_bs1)

    aln1_tmp.release()
    gb1_pool.release()
    x_pool.release()

    # ---------------- qkv + chunked collective ----------------
    # 4 AllGather chunks, one per 4-head group: chunk c carries k^T feature
    # rows [256c:256c+256] and v columns [256c:256c+256], so attention on
    # head-pairs 2c,2c+1 can start while later chunks are still in flight.
    kv_pool = pool("kv_pool")
    kT_loc = kv_pool.tile([128, 8 * T], BF16, name="kT_loc")
    v_loc = kv_pool.tile([128, 4 * D], BF16, name="v_loc")

    NCH = 4
    kv_ins = [dram.tile([512, T], BF16, name=f"kv_in{c}") for c in range(NCH)]
    kv_outs = [dram.tile([GROUP, 512, T], BF16, name=f"kv_out{c}") for c in range(NCH)]

    def evict(dst, src):
        # PSUM->SBUF bf16 eviction on the pre-attention-idle ACT engine
        if sim_safe:
            nc.vector.tensor_copy(dst, src)
        else:
            nc.scalar.activation(dst, src, AF.Copy)

    for c in range(NCH):
        # k^T feature M-tiles for heads 4c..4c+3
        for ml in range(2):
            m = 8 + 2 * c + ml
            ps = mm_ps.tile([128, T], F32, name="kps", tag="mm")
            for k in range(8):
                nc.tensor.matmul(ps[:], qkvw_sb[:, 3072 * k + 128 * m: 3072 * k + 128 * (m + 1)],
                                 h1_sb[:, k * T:(k + 1) * T],
                                 start=(k == 0), stop=(k == 7))
            evict(kT_loc[:, (m - 8) * T:(m - 7) * T], ps[:])
        # v quarter c ([tokens, 256 features]), token M-tiles
        for mt in range(4):
            ps = mm_ps.tile([128, 256], F32, name="vps", tag="mm")
            for k in range(8):
                nc.tensor.matmul(
                    ps[:],
                    h1_sb[:, k * T + 128 * mt: k * T + 128 * (mt + 1)],
                    qkvw_sb[:, 3072 * k + 2048 + 256 * c: 3072 * k + 2048 + 256 * (c + 1)],
                    start=(k == 0), stop=(k == 7))
            evict(v_loc[:, 1024 * mt + 256 * c: 1024 * mt + 256 * (c + 1)], ps[:])
        # bounce writes + collective for this chunk
        for ml in range(2):
            nc.sync.dma_start(kv_ins[c][128 * ml:128 * (ml + 1), :],
                              kT_loc[:, (2 * c + ml) * T:(2 * c + ml + 1) * T])
        vdst = kv_ins[c][256:512, :].rearrange("r (two f) -> (r two) f", two=2)
        nc.sync.dma_start(vdst.rearrange("(m p) f -> p m f", m=4),
                          v_loc.rearrange("p (m f) -> p m f", m=4)[:, :, 256 * c:256 * (c + 1)])
        nc.gpsimd.collective_compute(
            "AllGather",
            ALU.bypass,
            replica_groups=[[0, 1, 2, 3], [4, 5, 6, 7]],
            ins=[kv_ins[c][:]],
            outs=[kv_outs[c][:]],
        )
    kv_pool.release()

    # right-side carries for the attention phase
    gb2_pool = pool("gb2_pool", side="right")
    gb2_sb = gb2_pool.tile([128, 16 * T], BF16, name="gb2_sb")
    oT_pool = pool("oT_pool", side="right")
    oT_sb = oT_pool.tile([128, 8 * T], BF16, name="oT_sb")
    q_pool = pool("q_pool", side="right")
    qT_sb = q_pool.tile([128, 8 * T], BF16, name="qT_sb")

    # q^T (feature M-tiles 0..7), overlaps with collective
    for m in range(8):
        ps = mm_ps.tile([128, T], F32, name="qps", tag="mm")
        for k in range(8):
            nc.tensor.matmul(ps[:], qkvw_sb[:, 3072 * k + 128 * m: 3072 * k + 128 * (m + 1)],
                             h1_sb[:, k * T:(k + 1) * T],
                             start=(k == 0), stop=(k == 7))
        nc.vector.tensor_copy(qT_sb[:, m * T:(m + 1) * T], ps[:])

    # gb2 projection, overlaps with collective (DVE eviction: ACT must be
    # free to load the Exp table before the first attention tile lands)
    for m in range(16):
        gb_mtile(p2w_sb, p2b_sb, gb2_sb, m, mm_ps, evict_act=False)

    h1_pool.release()
    qkvw_pool.release()
    proj_pool.release()
    cond_pool.release()
    mm_ps.release()
    ps_a.release()

    # ---------------- attention ----------------
    att_pool = pool("att_pool")
    kT_full = att_pool.tile([128, 8 * S], BF16, name="kT_full")
    VW = DH + 1  # 65: per-head V columns + ones column (softmax denominator)
    vext = att_pool.tile([128, 16 * H * VW], BF16, name="vext")
    vext_v = vext.rearrange("p (c h m) -> p c h m", c=16, m=VW)
    nc.vector.memset(vext_v[:, :, :, DH:DH + 1], 1.0)

    def readback_chunk(c):
        for fl in range(2):
            f = 2 * c + fl
            for r in range(GROUP):
                nc.sync.dma_start(kT_full[:, 2048 * f + 512 * r: 2048 * f + 512 * (r + 1)],
                                  kv_outs[c][r, 128 * fl:128 * (fl + 1), :])
        for r in range(GROUP):
            vch = kv_outs[c][r, 256:512, :].rearrange("q (two f) -> (q two) f", two=2)
            for lc in range(4):
                c2 = 4 * r + lc
                src = vch[128 * lc:128 * (lc + 1), :].rearrange("t (h d) -> t h d", d=DH)
                # SWDGE queue: keeps vext readbacks off the sync DMA queues so
                # they don't serialize behind later chunks' waits
                nc.gpsimd.dma_start(vext_v[:, c2, 4 * c:4 * (c + 1), 0:DH], src)

    p_pool = pool("p_pool", bufs=4)
    norm_pool = pool("norm_pool", bufs=2)
    sc_ps = pool("sc_ps", bufs=2, space="PSUM")     # [128,1024] = 2 banks each
    o_ps_pool = pool("o_ps", bufs=2, space="PSUM")

    norm_pending = []

    def after_av(pv_hp, o_tiles):
        # Part A: evict raw o^T (freeing o psum quickly) and take the
        # reciprocal of the denominator rows straight out of PSUM; the
        # PE-side broadcast runs a pair later via flush_norm.
        recs = []
        for hh in range(2):
            nc.vector.tensor_copy(oT_sb[64 * hh:64 * (hh + 1), pv_hp * T:(pv_hp + 1) * T],
                                  o_tiles[hh][0:DH, :])
            den = norm_pool.tile([1, T], F32, name="den", tag="den", bufs=2)
            nc.vector.tensor_copy(den[:], o_tiles[hh][DH:DH + 1, :])
            rec = norm_pool.tile([1, T], F32, name="rec", tag="rec", bufs=2)
            if sim_safe:
                nc.vector.reciprocal(rec[:], den[:])
            else:
                nc.vector.reciprocal_approx_fast(rec[:], den[:])
            recs.append(rec)
        norm_pending.append((pv_hp, recs))

    def flush_norm():
        for (php, recs) in norm_pending:
            rb = o_ps_pool.tile([128, T], F32, name="rb", tag="rb", bufs=2)
            nc.tensor.matmul(rb[:], selA[:], recs[0][:], start=True, stop=False)
            nc.tensor.matmul(rb[:], selB[:], recs[1][:], start=False, stop=True)
            rb_sb = norm_pool.tile([128, T], BF16, name="rb_sb", tag="rbs")
            nc.vector.tensor_copy(rb_sb[:], rb[:])
            osl = oT_sb[:, php * T:(php + 1) * T]
            nc.vector.tensor_mul(osl, osl, rb_sb[:])
        norm_pending.clear()

    prev = None
    for hp in range(8):
        if hp % 2 == 0:
            readback_chunk(hp // 2)
        p_tiles = [p_pool.tile([128, 16 * T], BF16, name=f"pt{hh}", tag="p") for hh in range(2)]
        q_h = [qT_sb[64 * hh:64 * (hh + 1), hp * T:(hp + 1) * T] for hh in range(2)]
        o_tiles = None
        if prev is not None:
            o_tiles = [o_ps_pool.tile([128, T], F32, name="o_ps", tag="o") for _ in range(2)]
        # 8 groups: scores for chunks (2m2, 2m2+1) of both heads, interleaved
        # with 4 AV matmuls of the previous pair so PE work overlaps ACT exp.
        for m2 in range(8):
            scts = [sc_ps.tile([128, 1024], F32, name="sct", tag="s") for _ in range(2)]
            for half in range(2):
                m = 2 * m2 + half
                for hh in range(2):
                    rows = slice(64 * hh, 64 * (hh + 1))
                    nc.tensor.matmul(
                        scts[hh][:, 512 * half:512 * (half + 1)],
                        kT_full[rows, 2048 * hp + 128 * m: 2048 * hp + 128 * (m + 1)],
                        q_h[hh],
                        start=True, stop=True)
            if prev is not None:
                pv_tiles, pv_hp = prev
                for hh in range(2):
                    h = 2 * pv_hp + hh
                    for half in range(2):
                        cc = 2 * m2 + half
                        nc.tensor.matmul(
                            o_tiles[hh][0:VW, :],
                            vext[:, VW * (16 * cc + h): VW * (16 * cc + h) + VW],
                            pv_tiles[hh][:, cc * T:(cc + 1) * T],
                            start=(cc == 0), stop=(cc == 15))
            for hh in range(2):
                nc.scalar.activation(p_tiles[hh][:, 2 * m2 * T:(2 * m2 + 2) * T],
                                     scts[hh][:], AF.Exp)
        flush_norm()
        if prev is not None:
            after_av(prev[1], o_tiles)
        prev = (p_tiles, hp)

    # tail: AV + normalize for the last pair
    pv_tiles, pv_hp = prev
    o_tiles = [o_ps_pool.tile([128, T], F32, name="o_ps", tag="o") for _ in range(2)]
    for cc in range(16):
        for hh in range(2):
            h = 2 * pv_hp + hh
            nc.tensor.matmul(
                o_tiles[hh][0:VW, :],
                vext[:, VW * (16 * cc + h): VW * (16 * cc + h) + VW],
                pv_tiles[hh][:, cc * T:(cc + 1) * T],
                start=(cc == 0), stop=(cc == 15))
    flush_norm()
    after_av(pv_hp, o_tiles)
    flush_norm()

    o_ps_pool.release()
    sc_ps.release()
    norm_pool.release()
    p_pool.release()
    att_pool.release()
    q_pool.release()

    # ---------------- attn_out + residual + AdaLN2 stats ----------------
    ps_b = pool("ps_b", bufs=2, space="PSUM")
    mm_ps2 = pool("mm_ps2", bufs=3, space="PSUM")

    aln2_tmp = pool("aln2_tmp")
    src2_bf = aln2_tmp.tile([128, 8 * T], BF16, name="src2_bf")

    h2_pool = pool("h2_pool")
    h2_sb = h2_pool.tile([128, 8 * T], BF16, name="h2_sb")

    w1_pool = pool("w1_pool")
    w1_sb = w1_pool.tile([128, 8 * FF], BF16, name="w1_sb")

    wo_pool = pool("wo_pool")
    wo_sb = wo_pool.tile([128, 8 * D], BF16, name="wo_sb")
    for a in range(8):
        nc.sync.dma_start(wo_sb[:, 1024 * a:1024 * (a + 1)], wo[128 * a:128 * (a + 1), :])

    xre_pool = pool("xre_pool", side="right")
    xre_sb = xre_pool.tile([128, 8 * T], F32, name="xre_sb")
    for a in range(8):
        nc.sync.dma_start(xre_sb[:, T * a:T * (a + 1)], xT[128 * a:128 * (a + 1), :])

    for a in range(8):
        nc.sync.dma_start(w1_sb[:, 4096 * a:4096 * (a + 1)], w1[128 * a:128 * (a + 1), :])

    sums2 = ps_b.tile([1, T], F32, name="sums2", tag="st")
    sumsq2 = ps_b.tile([1, T], F32, name="sumsq2", tag="st")
    for m in range(8):
        sl = slice(m * T, (m + 1) * T)
        ps = mm_ps2.tile([128, T], F32, name="aops", tag="mm")
        for k in range(8):
            nc.tensor.matmul(ps[:], wo_sb[:, 1024 * k + 128 * m: 1024 * k + 128 * (m + 1)],
                             oT_sb[:, k * T:(k + 1) * T],
                             start=(k == 0), stop=(k == 7))
        nc.vector.tensor_add(x1t_sb[:, sl], ps[:], xre_sb[:, sl])
        # AdaLN2 stats chase the wo m-tiles
        nc.vector.tensor_copy(src2_bf[:, sl], x1t_sb[:, sl])
        sqj = work.tile([128, T], BF16, name="sqj2", tag="wk")
        nc.vector.tensor_mul(sqj[:], src2_bf[:, sl], src2_bf[:, sl])
        nc.tensor.matmul(sums2[:], ones_col_bf[:], src2_bf[:, sl],
                         start=(m == 0), stop=(m == 7))
        nc.tensor.matmul(sumsq2[:], ones_col_bf[:], sqj[:],
                         start=(m == 0), stop=(m == 7))
    wo_pool.release()
    xre_pool.release()
    oT_pool.release()

    mu_bs2, rs_bs2 = stats_finish(sums2, sumsq2, ps_b)
    for j in range(8):
        normalize_tile(j, src2_bf, gb2_sb, h2_sb, mu_bs2, rs_bs2)
    gb2_pool.release()

    # ---------------- FFN ----------------
    g_pool = pool("g_pool")
    g_sb = g_pool.tile([128, 32 * T], BF16, name="g_sb")

    for m in range(32):
        ps = mm_ps2.tile([128, T], F32, name="f1ps", tag="mm")
        for k in range(8):
            nc.tensor.matmul(ps[:], w1_sb[:, 4096 * k + 128 * m: 4096 * k + 128 * (m + 1)],
                             h2_sb[:, k * T:(k + 1) * T],
                             start=(k == 0), stop=(k == 7))
        if sim_safe:
            # sim has no Gelu: x*sigmoid(1.702x) approximation
            u = work.tile([128, T], F32, name="u", tag="wk32f")
            nc.vector.tensor_scalar_add(u[:], ps[:], b1_sb[:, m:m + 1])
            sg = work.tile([128, T], BF16, name="sg", tag="wk")
            nc.scalar.activation(sg[:], u[:], AF.Sigmoid, scale=1.702)
            nc.vector.tensor_mul(g_sb[:, m * T:(m + 1) * T], u[:], sg[:])
        else:
            nc.scalar.activation(g_sb[:, m * T:(m + 1) * T], ps[:], AF.Gelu,
                                 bias=b1_sb[:, m:m + 1], scale=1.0)
    mm_ps2.release()
    ps_b.release()

    # ffn2: k-outer, stream w2 k-tiles; two m-halves so the first half's
    # evictions + output stores overlap the second half's matmuls
    w2_pool = pool("w2_pool", bufs=4)
    ff2_ps = pool("ff2_ps", bufs=1, space="PSUM")
    out_pool0 = pool("out_pool0")
    out_sb = out_pool0.tile([128, 8 * T], F32, name="out_sb")
    for half in range(2):
        o2 = [ff2_ps.tile([128, T], F32, name=f"ff2_{m}", tag=f"ff2_{m}") for m in range(4)]
        for k in range(32):
            w2t = w2_pool.tile([128, 512], BF16, name="w2t", tag="w2t")
            nc.sync.dma_start(w2t[:], w2[128 * k: 128 * (k + 1), 512 * half:512 * (half + 1)])
            for m in range(4):
                nc.tensor.matmul(o2[m][:], w2t[:, 128 * m: 128 * (m + 1)],
                                 g_sb[:, k * T:(k + 1) * T],
                                 start=(k == 0), stop=(k == 31))
        for m in range(4):
            gm = 4 * half + m
            nc.vector.scalar_tensor_tensor(out_sb[:, gm * T:(gm + 1) * T], o2[m][:],
                                           b2_sb[:, gm:gm + 1], x1t_sb[:, gm * T:(gm + 1) * T],
                                           op0=ALU.add, op1=ALU.add)
            nc.sync.dma_start(out_d[128 * gm:128 * (gm + 1), :],
                              out_sb[:, T * gm:T * (gm + 1)])

    out_pool0.release()
    w2_pool.release()
    g_pool.release()
    w1_pool.release()
    h2_pool.release()
    aln2_tmp.release()
    ff2_ps.release()
    x1_pool.release()
    small.release()
    work.release()
    const.release()
    dram.release()


def _bf16(a):
    return np.ascontiguousarray(a).astype(ml_dtypes.bfloat16)


def _prep_maps(x, cond, p1_w, p1_b, qkv_w, attn_out_w, p2_w, p2_b,
               ffn_w1, ffn_b1, ffn_w2, ffn_b2):
    x = np.asarray(x, np.float32)
    cond = np.asarray(cond, np.float32)
    qkv_mod = np.asarray(qkv_w, np.float32).copy()
    qkv_mod[:, :D] *= DH ** -0.5                      # fold 1/sqrt(d) into q
    p1b_mod = np.asarray(p1_b, np.float32).copy()
    p1b_mod[:D] += 1.0                                # fold AdaLN "+1" into gamma bias
    p2b_mod = np.asarray(p2_b, np.float32).copy()
    p2b_mod[:D] += 1.0

    shared = {
        "p1w": _bf16(p1_w),
        "p1b": np.ascontiguousarray(p1b_mod.reshape(16, 128).T, np.float32),
        "qkvw": _bf16(qkv_mod),
        "wo": _bf16(attn_out_w),
        "p2w": _bf16(p2_w),
        "p2b": np.ascontiguousarray(p2b_mod.reshape(16, 128).T, np.float32),
        "w1": _bf16(ffn_w1),
        "b1": np.ascontiguousarray(np.asarray(ffn_b1, np.float32).reshape(32, 128).T,
                                   np.float32),
        "w2": _bf16(ffn_w2),
        "b2": np.ascontiguousarray(np.asarray(ffn_b2, np.float32).reshape(8, 128).T,
                                   np.float32),
    }
    in_maps = []
    for core in range(NCORES):
        b, r = core // GROUP, core % GROUP
        sl = slice(T * r, T * (r + 1))
        m = dict(shared)
        m["xT"] = np.ascontiguousarray(x[b, sl, :].T, np.float32)
        m["condT"] = _bf16(cond[b, sl, :].T)
        in_maps.append(m)
    return in_maps


def _get_nc():
    if "nc" not in _CACHE:
        _CACHE["nc"] = _build()
    return _CACHE["nc"]


def _install_ntff_hook():
    """This image's antenv lacks axon_hooks; recreate it (see trn_boot.py)."""
    import sys, types, ctypes, contextlib

    if "antenv.axon_hooks" in sys.modules:
        return
    mod = types.ModuleType("antenv.axon_hooks")
    state = {"hook": None}
    mod.set_axon_ntff_profile_hook = lambda h: state.__setitem__("hook", h)
    mod.get_axon_ntff_profile_hook = lambda: state["hook"]
    sys.modules["antenv.axon_hooks"] = mod
    try:
        import antenv
        antenv.axon_hooks = mod
    except ImportError:
        pass

    so_path = "/opt/axon/libaxon_pjrt.so"
    if not os.path.exists(so_path):
        return
    lib = ctypes.CDLL(so_path)
    if not hasattr(lib, "axon_start_nrt_profile"):
        return
    lib.axon_start_nrt_profile.argtypes = [ctypes.POINTER(ctypes.c_int64), ctypes.c_size_t]
    lib.axon_start_nrt_profile.restype = ctypes.c_int64
    lib.axon_stop_nrt_profile.argtypes = [ctypes.c_char_p]
    lib.axon_stop_nrt_profile.restype = ctypes.c_int64

    @contextlib.contextmanager
    def _hook(output_dir, device_ids):
        import jax
        jax.devices()
        if device_ids:
            ids = (ctypes.c_int64 * len(device_ids))(*device_ids)
            rc = lib.axon_start_nrt_profile(ids, len(device_ids))
        else:
            rc = lib.axon_start_nrt_profile(None, 0)
        if rc != 0:
            raise RuntimeError(f"axon_start_nrt_profile rc={rc}")
        try:
            yield
        finally:
            n = lib.axon_stop_nrt_profile(str(output_dir).encode())
            print(f"ntff profile: {n} file(s) -> {output_dir}")

    mod.set_axon_ntff_profile_hook(_hook)


def run(in_maps, trace=False, **kw):
    if trace:
        _install_ntff_hook()
    nc = _get_nc()
    return run_bass_kernel_spmd(nc, in_maps, core_ids=list(range(NCORES)),
                                trace=trace, **kw)


def kernel(**inputs):
    in_maps = _prep_maps(**inputs)
    res = run(in_maps).results
    out = np.empty((B, S, D), np.float32)
    for core in range(NCORES):
        b, r = core // GROUP, core % GROUP
        out[b, T * r: T * (r + 1), :] = res[core]["out"].T
    return out


# revision 37
# speedup vs baseline: 1.1522x; 1.1514x over previous
# Distributed Bass kernel for nn_DecoderBlock (AdaLN decoder block) on 8 TRN2 cores.
#
# Sharding: core i -> (batch b = i//4, sequence quarter r = i%4, 512 tokens).
# Weights replicated (bf16). The only collective is a 4-rank AllGather of the
# local K^T / V slices per batch group.
#
# Layout convention: every on-chip activation is stored transposed,
# [features(partitions), tokens(free)], so each linear y = h @ W uses the
# weight (in,out) directly as matmul lhsT and needs no on-chip transposes.
# Host pre-transposes/shards x and cond, folds 1/sqrt(d) into the q columns
# of qkv_w and the AdaLN "+1" into the gamma half of p1_b/p2_b.
#
# Engine budget per phase (per core): pre-attention is PE-bound (~77us of
# matmul), attention is ACT-bound (16.8M softmax exps ~= 147us), FFN tail is
# PE-bound (~110us). AdaLN stats matmuls interleave with the gamma-half
# projection matmuls so the LayerNorm chain never serializes the PE; gb/k/v
# PSUM evictions run on the otherwise-idle ACT engine pre-attention; softmax
# normalization uses reciprocal_approx_fast straight off the PSUM denominator
# row plus a single K=2 selector-broadcast matmul per head pair.

import os

os.environ.setdefault("MYCRO_LOCAL_CACHE", "1")

import numpy as np
import ml_dtypes

import concourse.bass as bass
import concourse.mybir as mybir
import concourse.tile as tile
from concourse import bacc
from concourse.bass_utils import run_bass_kernel_spmd

F32 = mybir.dt.float32
BF16 = mybir.dt.bfloat16
AF = mybir.ActivationFunctionType
ALU = mybir.AluOpType

D = 1024        # d_model
DC = 512        # d_cond
H = 16          # heads
DH = 64         # head dim
FF = 4096       # ffn dim
T = 512         # tokens per core
S = 2048        # sequence length per batch
B = 2
NCORES = 8
GROUP = 4       # cores per batch group
EPS = 1e-5

_CACHE = {}


def _build(sim_safe=False):
    nc = bacc.Bacc(
        "TRN2",
        target_bir_lowering=False,
        debug=False,
        enable_asserts=False,
        num_devices=NCORES,
    )

    # ---- DRAM I/O ----
    xT = nc.dram_tensor("xT", [D, T], F32, kind="ExternalInput").ap()
    xbf = nc.dram_tensor("xbf", [D, T], BF16, kind="ExternalInput").ap()
    condT = nc.dram_tensor("condT", [DC, T], BF16, kind="ExternalInput").ap()
    p1w = nc.dram_tensor("p1w", [DC, 2 * D], BF16, kind="ExternalInput").ap()
    p1b = nc.dram_tensor("p1b", [128, 16], F32, kind="ExternalInput").ap()
    qkvw = nc.dram_tensor("qkvw", [D, 3 * D], BF16, kind="ExternalInput").ap()
    wo = nc.dram_tensor("wo", [D, D], BF16, kind="ExternalInput").ap()
    p2w = nc.dram_tensor("p2w", [DC, 2 * D], BF16, kind="ExternalInput").ap()
    p2b = nc.dram_tensor("p2b", [128, 16], F32, kind="ExternalInput").ap()
    w1 = nc.dram_tensor("w1", [D, FF], BF16, kind="ExternalInput").ap()
    b1 = nc.dram_tensor("b1", [128, 32], F32, kind="ExternalInput").ap()
    w2 = nc.dram_tensor("w2", [FF, D], BF16, kind="ExternalInput").ap()
    b2 = nc.dram_tensor("b2", [128, 8], F32, kind="ExternalInput").ap()
    out_d = nc.dram_tensor("out", [D, T], F32, kind="ExternalOutput").ap()

    with tile.TileContext(nc) as tc:
        _emit(nc, tc, xT, xbf, condT, p1w, p1b, qkvw, wo, p2w, p2b, w1, b1, w2, b2, out_d, sim_safe)

    nc.compile()
    return nc


def _emit(nc, tc, xT, xbf, condT, p1w, p1b, qkvw, wo, p2w, p2b, w1, b1, w2, b2, out_d, sim_safe):
    # Pool lifetimes follow a two-sided stack discipline (LIFO per side):
    # left = phase-nested pools, right = phase-crossing carries.
    def pool(name, bufs=1, space="SBUF", side=None):
        return tc.alloc_tile_pool(name=name, bufs=bufs, space=space, side=side)

    # ---------------- persistent pools ----------------
    const = pool("const")
    work = pool("work", bufs=4)            # [128,T] temporaries
    small = pool("small", bufs=4)          # [1,T] stats
    dram = pool("dram", bufs=1, space="DRAM")

    # right-side carries (alloc order = reverse of release order)
    x1_pool = pool("x1_pool", side="right")                    # lives to the end
    x1t_sb = x1_pool.tile([128, 8 * T], F32, name="x1t_sb")
    x_pool = pool("x_pool", side="right")                      # released after AdaLN1
    xbf_sb = x_pool.tile([128, 8 * T], BF16, name="xbf_sb")

    # ---------------- constants ----------------
    ones_col_bf = const.tile([128, 1], BF16, name="ones_col_bf")
    nc.vector.memset(ones_col_bf[:], 1.0)
    ones_row_f = const.tile([1, 128], F32, name="ones_row_f")
    nc.vector.memset(ones_row_f[:], 1.0)
    eps_t = const.tile([1, 1], F32, name="eps_t")
    nc.vector.memset(eps_t[:], EPS)

    # head-pair selectors: rb[p,:] = recA for p<64, recB for p>=64
    selA = const.tile([1, 128], BF16, name="selA")
    nc.vector.memset(selA[:], 0.0)
    nc.vector.memset(selA[0:1, 0:64], 1.0)
    selB = const.tile([1, 128], BF16, name="selB")
    nc.vector.memset(selB[:], 0.0)
    nc.vector.memset(selB[0:1, 64:128], 1.0)

    # rhs for HAM-warmer matmuls (keep the PE clock-gate open across
    # sub-window stalls: a [1,128] matmul costs ~53ns warm)
    warm_src = const.tile([128, 128], BF16, name="warm_src")
    nc.vector.memset(warm_src[:], 0.0)

    p1b_sb = const.tile([128, 16], F32, name="p1b_sb")
    nc.sync.dma_start(p1b_sb[:], p1b)
    p2b_sb = const.tile([128, 16], F32, name="p2b_sb")
    nc.sync.dma_start(p2b_sb[:], p2b)
    b1_sb = const.tile([128, 32], F32, name="b1_sb")
    nc.sync.dma_start(b1_sb[:], b1)
    b2_sb = const.tile([128, 8], F32, name="b2_sb")
    nc.sync.dma_start(b2_sb[:], b2)

    # ---------------- input loads (cond/p1w first: they gate the first ops) --
    cond_pool = pool("cond_pool")
    cond_sb = cond_pool.tile([128, 4 * T], BF16, name="cond_sb")
    for a in range(4):
        nc.scalar.dma_start(cond_sb[:, T * a:T * (a + 1)], condT[128 * a:128 * (a + 1), :])
    sc_sb = cond_pool.tile([128, 4 * T], BF16, name="sc_sb")

    proj_pool = pool("proj_pool")
    p1w_sb = proj_pool.tile([128, 4 * 2048], BF16, name="p1w_sb")
    for a in range(4):
        nc.sync.dma_start(p1w_sb[:, 2048 * a:2048 * (a + 1)], p1w[128 * a:128 * (a + 1), :])

    for a in range(8):
        nc.sync.dma_start(xbf_sb[:, T * a:T * (a + 1)], xbf[128 * a:128 * (a + 1), :])

    qkvw_pool = pool("qkvw_pool")
    qkvw_sb = qkvw_pool.tile([128, 8 * 3072], BF16, name="qkvw_sb")
    for a in range(8):
        nc.sync.dma_start(qkvw_sb[:, 3072 * a:3072 * (a + 1)], qkvw[128 * a:128 * (a + 1), :])

    p2w_sb = proj_pool.tile([128, 4 * 2048], BF16, name="p2w_sb")
    for a in range(4):
        nc.sync.dma_start(p2w_sb[:, 2048 * a:2048 * (a + 1)], p2w[128 * a:128 * (a + 1), :])

    ps_a = pool("ps_a", bufs=2, space="PSUM")       # stats + broadcast banks
    mm_ps = pool("mm_ps", bufs=3, space="PSUM")     # matmul eviction banks

    def warmers(ps_pool, rhs, n):
        """n tiny matmuls into a scratch PSUM bank; pure HAM-keepalive.
        The rhs choice doubles as a scheduling pin: the first warmer waits
        for it, so the burst lands where the producer finishes."""
        wt = ps_pool.tile([1, 128], F32, name="warm", tag="warm", bufs=1)
        for _ in range(n):
            nc.tensor.matmul(wt[:], ones_col_bf[:], rhs,
                             start=True, stop=True)

    # ---------------- AdaLN helpers ----------------
    def stats_finish(sums, sumsq, st_ps):
        """[1,T] PSUM sums -> broadcast bf16 mu/rs tiles (PE+ACT+DVE smalls)."""
        mu = small.tile([1, T], F32, name="mu", tag="sm")
        nc.vector.tensor_scalar_mul(mu[:], sums[:], 1.0 / D)
        musq = small.tile([1, T], F32, name="musq", tag="sm")
        nc.vector.tensor_mul(musq[:], mu[:], mu[:])
        var = small.tile([1, T], F32, name="var", tag="sm")
        nc.vector.scalar_tensor_tensor(var[:], sumsq[:], 1.0 / D, musq[:],
                                       op0=ALU.mult, op1=ALU.subtract)
        rs = small.tile([1, T], F32, name="rs", tag="sm")
        sd = small.tile([1, T], F32, name="sd", tag="sm")
        nc.scalar.activation(sd[:], var[:], AF.Sqrt, bias=eps_t[:], scale=1.0)
        if sim_safe:
            nc.vector.reciprocal(rs[:], sd[:])
        else:
            nc.vector.reciprocal_approx_fast(rs[:], sd[:])

        mu_b = st_ps.tile([128, T], F32, name="mu_b", tag="bc")
        nc.tensor.matmul(mu_b[:], ones_row_f[:], mu[:], start=True, stop=True)
        rs_b = st_ps.tile([128, T], F32, name="rs_b", tag="bc")
        nc.tensor.matmul(rs_b[:], ones_row_f[:], rs[:], start=True, stop=True)
        mu_bs = work.tile([128, T], BF16, name="mu_bs", tag="wk")
        nc.vector.tensor_copy(mu_bs[:], mu_b[:])
        rs_bs = work.tile([128, T], BF16, name="rs_bs", tag="wk")
        nc.vector.tensor_copy(rs_bs[:], rs_b[:])
        return mu_bs, rs_bs

    def normalize_tile(j, src_bf, gb_sb, h_sb, mu_bs, rs_bs):
        sl = slice(j * T, (j + 1) * T)
        t1 = work.tile([128, T], BF16, name="t1", tag="wk")
        nc.vector.tensor_sub(t1[:], src_bf[:, sl], mu_bs[:])
        t2 = work.tile([128, T], BF16, name="t2", tag="wk")
        nc.vector.tensor_mul(t2[:], t1[:], rs_bs[:])
        t3 = work.tile([128, T], BF16, name="t3", tag="wk")
        nc.vector.tensor_mul(t3[:], t2[:], gb_sb[:, sl])           # *(1+gamma)
        nc.vector.tensor_add(h_sb[:, sl], t3[:], gb_sb[:, (8 + j) * T:(9 + j) * T])

    def gb_mtile(w_sb, b_sb, gb_sb, m, m_ps, evict_act=True):
        ps = m_ps.tile([128, T], F32, name="gbps", tag="mm")
        for k in range(4):
            nc.tensor.matmul(ps[:], w_sb[:, 2048 * k + 128 * m: 2048 * k + 128 * (m + 1)],
                             sc_sb[:, k * T:(k + 1) * T],
                             start=(k == 0), stop=(k == 3))
        if evict_act and not sim_safe:
            nc.scalar.activation(gb_sb[:, m * T:(m + 1) * T], ps[:], AF.Identity,
                                 bias=b_sb[:, m:m + 1], scale=1.0)
        else:
            nc.vector.tensor_scalar_add(gb_sb[:, m * T:(m + 1) * T], ps[:],
                                        b_sb[:, m:m + 1])

    # ---------------- AdaLN 1 ----------------
    # silu(c) = c * sigmoid(c), per tile so proj matmuls start early
    sig_sb = cond_pool.tile([128, 4 * T], BF16, name="sig_sb")
    for a in range(4):
        sl = slice(T * a, T * (a + 1))
        nc.scalar.activation(sig_sb[:, sl], cond_sb[:, sl], AF.Sigmoid)
        nc.vector.tensor_mul(sc_sb[:, sl], cond_sb[:, sl], sig_sb[:, sl])

    h1_pool = pool("h1_pool")
    h1_sb = h1_pool.tile([128, 8 * T], BF16, name="h1_sb")

    gb1_pool = pool("gb1_pool")
    gb1_sb = gb1_pool.tile([128, 16 * T], BF16, name="gb1_sb")

    aln1_tmp = pool("aln1_tmp")

    sums1 = ps_a.tile([1, T], F32, name="sums1", tag="st")
    sumsq1 = ps_a.tile([1, T], F32, name="sumsq1", tag="st")
    # DVE computes the stats inputs while the PE streams gb1; sq tiles live
    # in their own ring so all 8 survive until the dense stats-MM block.
    sq1 = aln1_tmp.tile([128, 8 * T], BF16, name="sq1")
    for j in range(8):
        sl = slice(j * T, (j + 1) * T)
        nc.vector.tensor_mul(sq1[:, sl], xbf_sb[:, sl], xbf_sb[:, sl])
    # warm the PE clock-gate during the input-DMA wait; the stats block is
    # emitted first so the scheduler slots it as soon as the sq tiles land,
    # and gb1 interleaves gamma/beta pairs so normalize_tile(j) unblocks as
    # soon as pair j and the stats are done
    warmers(ps_a, warm_src[:, 0:128], 72)
    for j in range(8):
        sl = slice(j * T, (j + 1) * T)
        nc.tensor.matmul(sums1[:], ones_col_bf[:], xbf_sb[:, sl],
                         start=(j == 0), stop=(j == 7))
        nc.tensor.matmul(sumsq1[:], ones_col_bf[:], sq1[:, sl],
                         start=(j == 0), stop=(j == 7))
    for j in range(8):
        gb_mtile(p1w_sb, p1b_sb, gb1_sb, j, mm_ps)
        gb_mtile(p1w_sb, p1b_sb, gb1_sb, 8 + j, mm_ps)
    mu_bs1, rs_bs1 = stats_finish(sums1, sumsq1, ps_a)
    for j in range(8):
        normalize_tile(j, xbf_sb, gb1_sb, h1_sb, mu_bs1, rs_bs1)

    aln1_tmp.release()
    gb1_pool.release()
    x_pool.release()

    # ---------------- qkv + chunked collective ----------------
    # 4 AllGather chunks, one per 4-head group: chunk c carries k^T feature
    # rows [256c:256c+256] and v columns [256c:256c+256], so attention on
    # head-pairs 2c,2c+1 can start while later chunks are still in flight.
    kv_pool = pool("kv_pool")
    kT_loc = kv_pool.tile([128, 8 * T], BF16, name="kT_loc")
    v_loc = kv_pool.tile([128, 4 * D], BF16, name="v_loc")

    NCH = 4
    kv_ins = [dram.tile([512, T], BF16, name=f"kv_in{c}") for c in range(NCH)]
    kv_outs = [dram.tile([GROUP, 512, T], BF16, name=f"kv_out{c}") for c in range(NCH)]

    def evict(dst, src):
        # k/v evictions gate the AllGather: DVE reaches them ~20us before the
        # ACT queue (which is busy with the gb1 evictions) would
        nc.vector.tensor_copy(dst, src)

    for c in range(NCH):
        # k^T feature M-tiles for heads 4c..4c+3
        for ml in range(2):
            m = 8 + 2 * c + ml
            ps = mm_ps.tile([128, T], F32, name="kps", tag="mm")
            for k in range(8):
                nc.tensor.matmul(ps[:], qkvw_sb[:, 3072 * k + 128 * m: 3072 * k + 128 * (m + 1)],
                                 h1_sb[:, k * T:(k + 1) * T],
                                 start=(k == 0), stop=(k == 7))
            evict(kT_loc[:, (m - 8) * T:(m - 7) * T], ps[:])
        # v quarter c ([tokens, 256 features]), token M-tiles
        for mt in range(4):
            ps = mm_ps.tile([128, 256], F32, name="vps", tag="mm")
            for k in range(8):
                nc.tensor.matmul(
                    ps[:],
                    h1_sb[:, k * T + 128 * mt: k * T + 128 * (mt + 1)],
                    qkvw_sb[:, 3072 * k + 2048 + 256 * c: 3072 * k + 2048 + 256 * (c + 1)],
                    start=(k == 0), stop=(k == 7))
            evict(v_loc[:, 1024 * mt + 256 * c: 1024 * mt + 256 * (c + 1)], ps[:])
        # bounce writes + collective for this chunk
        for ml in range(2):
            nc.scalar.dma_start(kv_ins[c][128 * ml:128 * (ml + 1), :],
                                kT_loc[:, (2 * c + ml) * T:(2 * c + ml + 1) * T])
        # v chunk as 4 rectangular DMAs: token block mt -> rows
        # 256+128*(mt%2), cols 256*(mt//2)
        for mt in range(4):
            nc.scalar.dma_start(
                kv_ins[c][256 + 128 * (mt % 2):256 + 128 * (mt % 2) + 128,
                          256 * (mt // 2):256 * (mt // 2) + 256],
                v_loc[:, 1024 * mt + 256 * c:1024 * mt + 256 * (c + 1)])
        nc.gpsimd.collective_compute(
            "AllGather",
            ALU.bypass,
            replica_groups=[[0, 1, 2, 3], [4, 5, 6, 7]],
            ins=[kv_ins[c][:]],
            outs=[kv_outs[c][:]],
        )
    kv_pool.release()

    # right-side carries for the attention phase
    gb2_pool = pool("gb2_pool", side="right")
    gb2_sb = gb2_pool.tile([128, 16 * T], BF16, name="gb2_sb")
    oT_pool = pool("oT_pool", side="right")
    oT_sb = oT_pool.tile([128, 8 * T], BF16, name="oT_sb")
    q_pool = pool("q_pool", side="right")
    qT_sb = q_pool.tile([128, 8 * T], BF16, name="qT_sb")

    # q^T (feature M-tiles 0..7), overlaps with collective
    for m in range(8):
        ps = mm_ps.tile([128, T], F32, name="qps", tag="mm")
        for k in range(8):
            nc.tensor.matmul(ps[:], qkvw_sb[:, 3072 * k + 128 * m: 3072 * k + 128 * (m + 1)],
                             h1_sb[:, k * T:(k + 1) * T],
                             start=(k == 0), stop=(k == 7))
        nc.vector.tensor_copy(qT_sb[:, m * T:(m + 1) * T], ps[:])

    # gb2 projection, overlaps with collective (DVE eviction: ACT must be
    # free to load the Exp table before the first attention tile lands)
    for m in range(16):
        gb_mtile(p2w_sb, p2b_sb, gb2_sb, m, mm_ps, evict_act=False)

    h1_pool.release()
    qkvw_pool.release()
    proj_pool.release()
    cond_pool.release()
    mm_ps.release()
    ps_a.release()

    # ---------------- attention ----------------
    att_pool = pool("att_pool")
    kT_full = att_pool.tile([128, 8 * S], BF16, name="kT_full")
    VW = DH + 1  # 65: per-head V columns + ones column (softmax denominator)
    vext = att_pool.tile([128, 16 * H * VW], BF16, name="vext")
    vext_v = vext.rearrange("p (c h m) -> p c h m", c=16, m=VW)
    nc.vector.memset(vext_v[:, :, :, DH:DH + 1], 1.0)

    def readback_chunk(c):
        for fl in range(2):
            f = 2 * c + fl
            for r in range(GROUP):
                nc.sync.dma_start(kT_full[:, 2048 * f + 512 * r: 2048 * f + 512 * (r + 1)],
                                  kv_outs[c][r, 128 * fl:128 * (fl + 1), :])
        for r in range(GROUP):
            for lc in range(4):
                c2 = 4 * r + lc
                src = kv_outs[c][r, 256 + 128 * (lc % 2):256 + 128 * (lc % 2) + 128,
                                 256 * (lc // 2):256 * (lc // 2) + 256]
                src = src.rearrange("t (h d) -> t h d", d=DH)
                # SWDGE queue: keeps vext readbacks off the sync DMA queues so
                # they don't serialize behind later chunks' waits
                nc.gpsimd.dma_start(vext_v[:, c2, 4 * c:4 * (c + 1), 0:DH], src)

    p_pool = pool("p_pool", bufs=4)
    norm_pool = pool("norm_pool", bufs=2)
    warm_ps = pool("warm_ps", bufs=1, space="PSUM")
    sc_ps = pool("sc_ps", bufs=2, space="PSUM")     # [128,1024] = 2 banks each
    o_ps_pool = pool("o_ps", bufs=2, space="PSUM")

    norm_pending = []

    def after_av(pv_hp, o_tiles):
        # Part A: evict raw o^T (freeing o psum quickly) and take the
        # reciprocal of the denominator rows straight out of PSUM; the
        # PE-side broadcast runs a pair later via flush_norm.
        recs = []
        for hh in range(2):
            nc.vector.tensor_copy(oT_sb[64 * hh:64 * (hh + 1), pv_hp * T:(pv_hp + 1) * T],
                                  o_tiles[hh][0:DH, :])
            den = norm_pool.tile([1, T], F32, name="den", tag="den", bufs=2)
            nc.vector.tensor_copy(den[:], o_tiles[hh][DH:DH + 1, :])
            rec = norm_pool.tile([1, T], F32, name="rec", tag="rec", bufs=2)
            if sim_safe:
                nc.vector.reciprocal(rec[:], den[:])
            else:
                nc.vector.reciprocal_approx_fast(rec[:], den[:])
            recb = norm_pool.tile([1, T], BF16, name="recb", tag="recb", bufs=2)
            nc.vector.tensor_copy(recb[:], rec[:])
            recs.append(recb)
        norm_pending.append((pv_hp, recs))

    def flush_norm():
        for (php, recs) in norm_pending:
            rb = o_ps_pool.tile([128, T], F32, name="rb", tag="rb", bufs=1)
            nc.tensor.matmul(rb[:], selA[:], recs[0][:], start=True, stop=False)
            nc.tensor.matmul(rb[:], selB[:], recs[1][:], start=False, stop=True)
            rb_sb = norm_pool.tile([128, T], BF16, name="rb_sb", tag="rbs")
            nc.vector.tensor_copy(rb_sb[:], rb[:])
            osl = oT_sb[:, php * T:(php + 1) * T]
            nc.vector.tensor_mul(osl, osl, rb_sb[:])
        norm_pending.clear()

    prev = None
    for hp in range(8):
        if hp % 2 == 0:
            readback_chunk(hp // 2)
        if hp == 0:
            # pre-warm the PE during the chunk-0 readback wait
            warmers(warm_ps, warm_src[:, 0:128], 32)
        p_tiles = [p_pool.tile([128, 16 * T], BF16, name=f"pt{hh}", tag="p") for hh in range(2)]
        q_h = [qT_sb[64 * hh:64 * (hh + 1), hp * T:(hp + 1) * T] for hh in range(2)]
        o_tiles = None
        if prev is not None:
            o_tiles = [o_ps_pool.tile([128, T], F32, name="o_ps", tag="o") for _ in range(2)]
        # 8 groups: scores for chunks (2m2, 2m2+1) of both heads, interleaved
        # with 4 AV matmuls of the previous pair so PE work overlaps ACT exp.
        for m2 in range(8):
            scts = [sc_ps.tile([128, 1024], F32, name="sct", tag="s") for _ in range(2)]
            if prev is not None:
                pv_tiles, pv_hp = prev
                for hh in range(2):
                    h = 2 * pv_hp + hh
                    for half in range(2):
                        cc = 2 * m2 + half
                        nc.tensor.matmul(
                            o_tiles[hh][0:VW, :],
                            vext[:, VW * (16 * cc + h): VW * (16 * cc + h) + VW],
                            pv_tiles[hh][:, cc * T:(cc + 1) * T],
                            start=(cc == 0), stop=(cc == 15))
            for half in range(2):
                m = 2 * m2 + half
                for hh in range(2):
                    rows = slice(64 * hh, 64 * (hh + 1))
                    nc.tensor.matmul(
                        scts[hh][:, 512 * half:512 * (half + 1)],
                        kT_full[rows, 2048 * hp + 128 * m: 2048 * hp + 128 * (m + 1)],
                        q_h[hh],
                        start=True, stop=True)
            for hh in range(2):
                nc.scalar.activation(p_tiles[hh][:, 2 * m2 * T:(2 * m2 + 2) * T],
                                     scts[hh][:], AF.Exp)
        flush_norm()
        if prev is not None:
            after_av(prev[1], o_tiles)
        prev = (p_tiles, hp)

    # preload the Sqrt and Gelu ACT spline tables while the PE drains the
    # attention tail, so the AdaLN2 chain and first Gelu skip the ~1.3us
    # table switch
    if not sim_safe:
        tpre = small.tile([1, 1], F32, name="tpre", tag="tpre", bufs=2)
        nc.scalar.activation(tpre[:], prev[0][1][0:1, 15 * T:15 * T + 1], AF.Sqrt)

    # tail: AV + normalize for the last pair
    pv_tiles, pv_hp = prev
    o_tiles = [o_ps_pool.tile([128, T], F32, name="o_ps", tag="o") for _ in range(2)]
    for cc in range(16):
        for hh in range(2):
            h = 2 * pv_hp + hh
            nc.tensor.matmul(
                o_tiles[hh][0:VW, :],
                vext[:, VW * (16 * cc + h): VW * (16 * cc + h) + VW],
                pv_tiles[hh][:, cc * T:(cc + 1) * T],
                start=(cc == 0), stop=(cc == 15))
    flush_norm()
    after_av(pv_hp, o_tiles)
    flush_norm()

    o_ps_pool.release()
    sc_ps.release()
    norm_pool.release()
    p_pool.release()
    att_pool.release()
    q_pool.release()

    # ---------------- attn_out + residual + AdaLN2 stats ----------------
    ps_b = pool("ps_b", bufs=2, space="PSUM")
    mm_ps2 = pool("mm_ps2", bufs=3, space="PSUM")

    aln2_tmp = pool("aln2_tmp")
    src2_bf = aln2_tmp.tile([128, 8 * T], BF16, name="src2_bf")

    h2_pool = pool("h2_pool")
    h2_sb = h2_pool.tile([128, 8 * T], BF16, name="h2_sb")

    w1_pool = pool("w1_pool")
    w1_sb = w1_pool.tile([128, 8 * FF], BF16, name="w1_sb")

    wo_pool = pool("wo_pool")
    wo_sb = wo_pool.tile([128, 8 * D], BF16, name="wo_sb")
    for a in range(8):
        nc.sync.dma_start(wo_sb[:, 1024 * a:1024 * (a + 1)], wo[128 * a:128 * (a + 1), :])

    xre_pool = pool("xre_pool", side="right")
    xre_sb = xre_pool.tile([128, 8 * T], F32, name="xre_sb")
    for a in range(8):
        nc.sync.dma_start(xre_sb[:, T * a:T * (a + 1)], xT[128 * a:128 * (a + 1), :])

    for a in range(8):
        nc.sync.dma_start(w1_sb[:, 4096 * a:4096 * (a + 1)], w1[128 * a:128 * (a + 1), :])

    sums2 = ps_b.tile([1, T], F32, name="sums2", tag="st")
    sumsq2 = ps_b.tile([1, T], F32, name="sumsq2", tag="st")
    # re-warm the PE across the attention-drain / wo-DMA boundary
    # (pinned on the hp6 oT columns so the burst lands at the drain)
    warmers(warm_ps, oT_sb[0:128, 6 * T:6 * T + 128], 40)
    for m in range(8):
        sl = slice(m * T, (m + 1) * T)
        ps = mm_ps2.tile([128, T], F32, name="aops", tag="mm")
        for k in range(8):
            nc.tensor.matmul(ps[:], wo_sb[:, 1024 * k + 128 * m: 1024 * k + 128 * (m + 1)],
                             oT_sb[:, k * T:(k + 1) * T],
                             start=(k == 0), stop=(k == 7))
        nc.vector.tensor_add(x1t_sb[:, sl], ps[:], xre_sb[:, sl])
        # AdaLN2 stats chase the wo m-tiles
        nc.vector.tensor_copy(src2_bf[:, sl], x1t_sb[:, sl])
        sqj = work.tile([128, T], BF16, name="sqj2", tag="wk")
        nc.vector.tensor_mul(sqj[:], src2_bf[:, sl], src2_bf[:, sl])
        nc.tensor.matmul(sums2[:], ones_col_bf[:], src2_bf[:, sl],
                         start=(m == 0), stop=(m == 7))
        nc.tensor.matmul(sumsq2[:], ones_col_bf[:], sqj[:],
                         start=(m == 0), stop=(m == 7))
    wo_pool.release()
    xre_pool.release()
    oT_pool.release()

    mu_bs2, rs_bs2 = stats_finish(sums2, sumsq2, ps_b)
    for j in range(8):
        normalize_tile(j, src2_bf, gb2_sb, h2_sb, mu_bs2, rs_bs2)
    gb2_pool.release()

    # ---------------- FFN ----------------
    g_pool = pool("g_pool")
    g_sb = g_pool.tile([128, 32 * T], BF16, name="g_sb")

    # keep the PE warm while the AdaLN2 mu/rs chain + normalize run on DVE/ACT
    warmers(warm_ps, src2_bf[0:128, 7 * T:7 * T + 128], 32)
    for m in range(32):
        ps = mm_ps2.tile([128, T], F32, name="f1ps", tag="mm")
        for k in range(8):
            nc.tensor.matmul(ps[:], w1_sb[:, 4096 * k + 128 * m: 4096 * k + 128 * (m + 1)],
                             h2_sb[:, k * T:(k + 1) * T],
                             start=(k == 0), stop=(k == 7))
        if sim_safe:
            # sim has no Gelu: x*sigmoid(1.702x) approximation
            u = work.tile([128, T], F32, name="u", tag="wk32f")
            nc.vector.tensor_scalar_add(u[:], ps[:], b1_sb[:, m:m + 1])
            sg = work.tile([128, T], BF16, name="sg", tag="wk")
            nc.scalar.activation(sg[:], u[:], AF.Sigmoid, scale=1.702)
            nc.vector.tensor_mul(g_sb[:, m * T:(m + 1) * T], u[:], sg[:])
        else:
            nc.scalar.activation(g_sb[:, m * T:(m + 1) * T], ps[:], AF.Gelu,
                                 bias=b1_sb[:, m:m + 1], scale=1.0)
    mm_ps2.release()
    ps_b.release()
    warm_ps.release()

    # ffn2: k-outer, stream w2 k-tiles; two m-halves so the first half's
    # evictions + output stores overlap the second half's matmuls
    w2_pool = pool("w2_pool", bufs=8)
    ff2_ps = pool("ff2_ps", bufs=1, space="PSUM")
    out_pool0 = pool("out_pool0")
    out_sb = out_pool0.tile([128, 8 * T], F32, name="out_sb")
    for half in range(2):
        o2 = [ff2_ps.tile([128, T], F32, name=f"ff2_{half}_{m}", tag=f"ff2_{half}_{m}")
              for m in range(4)]
        for k in range(32):
            w2t = w2_pool.tile([128, 512], BF16, name="w2t", tag="w2t")
            nc.sync.dma_start(w2t[:], w2[128 * k: 128 * (k + 1), 512 * half:512 * (half + 1)])
            for m in range(4):
                nc.tensor.matmul(o2[m][:], w2t[:, 128 * m: 128 * (m + 1)],
                                 g_sb[:, k * T:(k + 1) * T],
                                 start=(k == 0), stop=(k == 31))
        for m in range(4):
            gm = 4 * half + m
            nc.vector.scalar_tensor_tensor(out_sb[:, gm * T:(gm + 1) * T], o2[m][:],
                                           b2_sb[:, gm:gm + 1], x1t_sb[:, gm * T:(gm + 1) * T],
                                           op0=ALU.add, op1=ALU.add)
            # ACT's DMA queue: keeps stores from stalling the w2 prefetch
            # stream on the sync queue
            nc.scalar.dma_start(out_d[128 * gm:128 * (gm + 1), :],
                                out_sb[:, T * gm:T * (gm + 1)])

    out_pool0.release()
    w2_pool.release()
    g_pool.release()
    w1_pool.release()
    h2_pool.release()
    aln2_tmp.release()
    ff2_ps.release()
    x1_pool.release()
    small.release()
    work.release()
    const.release()
    dram.release()


def _bf16(a):
    return np.ascontiguousarray(a).astype(ml_dtypes.bfloat16)


def _prep_maps(x, cond, p1_w, p1_b, qkv_w, attn_out_w, p2_w, p2_b,
               ffn_w1, ffn_b1, ffn_w2, ffn_b2):
    x = np.asarray(x, np.float32)
    cond = np.asarray(cond, np.float32)
    qkv_mod = np.asarray(qkv_w, np.float32).copy()
    qkv_mod[:, :D] *= DH ** -0.5                      # fold 1/sqrt(d) into q
    p1b_mod = np.asarray(p1_b, np.float32).copy()
    p1b_mod[:D] += 1.0                                # fold AdaLN "+1" into gamma bias
    p2b_mod = np.asarray(p2_b, np.float32).copy()
    p2b_mod[:D] += 1.0

    shared = {
        "p1w": _bf16(p1_w),
        "p1b": np.ascontiguousarray(p1b_mod.reshape(16, 128).T, np.float32),
        "qkvw": _bf16(qkv_mod),
        "wo": _bf16(attn_out_w),
        "p2w": _bf16(p2_w),
        "p2b": np.ascontiguousarray(p2b_mod.reshape(16, 128).T, np.float32),
        "w1": _bf16(ffn_w1),
        "b1": np.ascontiguousarray(np.asarray(ffn_b1, np.float32).reshape(32, 128).T,
                                   np.float32),
        "w2": _bf16(ffn_w2),
        "b2": np.ascontiguousarray(np.asarray(ffn_b2, np.float32).reshape(8, 128).T,
                                   np.float32),
    }
    in_maps = []
    for core in range(NCORES):
        b, r = core // GROUP, core % GROUP
        sl = slice(T * r, T * (r + 1))
        m = dict(shared)
        m["xT"] = np.ascontiguousarray(x[b, sl, :].T, np.float32)
        m["xbf"] = _bf16(x[b, sl, :].T)
        m["condT"] = _bf16(cond[b, sl, :].T)
        in_maps.append(m)
    return in_maps


def _get_nc():
    if "nc" not in _CACHE:
        _CACHE["nc"] = _build()
    return _CACHE["nc"]


def _install_ntff_hook():
    """This image's antenv lacks axon_hooks; recreate it (see trn_boot.py)."""
    import sys, types, ctypes, contextlib

    if "antenv.axon_hooks" in sys.modules:
        return
    mod = types.ModuleType("antenv.axon_hooks")
    state = {"hook": None}
    mod.set_axon_ntff_profile_hook = lambda h: state.__setitem__("hook", h)
    mod.get_axon_ntff_profile_hook = lambda: state["hook"]
    sys.modules["antenv.axon_hooks"] = mod
    try:
        import antenv
        antenv.axon_hooks = mod
    except ImportError:
        pass

    so_path = "/opt/axon/libaxon_pjrt.so"
    if not os.path.exists(so_path):
        return
    lib = ctypes.CDLL(so_path)
    if not hasattr(lib, "axon_start_nrt_profile"):
        return
    lib.axon_start_nrt_profile.argtypes = [ctypes.POINTER(ctypes.c_int64), ctypes.c_size_t]
    lib.axon_start_nrt_profile.restype = ctypes.c_int64
    lib.axon_stop_nrt_profile.argtypes = [ctypes.c_char_p]
    lib.axon_stop_nrt_profile.restype = ctypes.c_int64

    @contextlib.contextmanager
    def _hook(output_dir, device_ids):
        import jax
        jax.devices()
        if device_ids:
            ids = (ctypes.c_int64 * len(device_ids))(*device_ids)
            rc = lib.axon_start_nrt_profile(ids, len(device_ids))
        else:
            rc = lib.axon_start_nrt_profile(None, 0)
        if rc != 0:
            raise RuntimeError(f"axon_start_nrt_profile rc={rc}")
        try:
            yield
        finally:
            n = lib.axon_stop_nrt_profile(str(output_dir).encode())
            print(f"ntff profile: {n} file(s) -> {output_dir}")

    mod.set_axon_ntff_profile_hook(_hook)


def run(in_maps, trace=False, **kw):
    if trace:
        _install_ntff_hook()
    nc = _get_nc()
    return run_bass_kernel_spmd(nc, in_maps, core_ids=list(range(NCORES)),
                                trace=trace, **kw)


def kernel(**inputs):
    in_maps = _prep_maps(**inputs)
    res = run(in_maps).results
    out = np.empty((B, S, D), np.float32)
    for core in range(NCORES):
        b, r = core // GROUP, core % GROUP
        out[b, T * r: T * (r + 1), :] = res[core]["out"].T
    return out
